# revision 34
# baseline (speedup 1.0000x reference)
"""Trainium2 Bass kernel for the entropy-regularized knapsack CVX loss.

Math: with e = x / (||x||_2 * TAU), the per-row solution of
    max e@z + EPS*sum(entr(z))  s.t. 0<=z<=1, sum z = K
is p_i = min(1, exp((e_i - nu)/EPS - 1)) with nu s.t. sum_i p_i = K.
Since |e_i| <= 1 (Cauchy-Schwarz) and n = 8192 >> K*e^2, the min(1,.)
clamp is never active at the optimum, so p = K * softmax(e) and
loss = mean(-log(K*exp(e_y)/s + 1e-8)) with s = sum_j exp(e_j).

Key reduction: ||e||_2 = 1/TAU = 1, so the 2nd-order Taylor expansion of
s around 0 is UNCONDITIONALLY accurate:
    s = sum exp(e_j) = N + sum e_j + 0.5*sum e_j^2 + R,
    |R| <= e/6 * (sum e_j^2)^{3/2} ~ 0.45 abs  (vs s ~ N = 8192),
i.e. rel err <= 5.6e-5 for ANY row; sum e_j^2 = 1 exactly.  The linear
term (~1e-4 relative for real data, <= 1.1% worst-case) is dropped --
validated: loss rel err vs reference ~1e-7 (tolerance 2e-2).

So the DEVICE only needs the per-row sum of squares S2 = sum_j x_ij^2
(norm and quadratic term in one).  Host does the O(B) rest: gather
x[r, y[r]], p_y = K*exp(x_y/sqrt(S2))/(N + 0.5), loss mean.

Device kernel (data-parallel over 8 cores, 1024 rows each, fp8 input):
three engines square-reduce disjoint column ranges in parallel so the
kernel rides the 8 MB/core fp8 DMA roofline (~23.5 us @ 358 GB/s):
  - ScalarE:  cols [0, A)        Square activation + fused accum
  - VectorE:  cols [A, A+D)      scalar_tensor_tensor (x*1)*x + accum
  - TensorE:  cols [A+D, 8192)   host-transposed 128-col chunks; for
    each 128-row block rb, matmul(lhsT=xT_chunk[:, rb], rhs=same)
    accumulates the Gram block of rows rb into PSUM bank rb; the
    diagonal (= sum of squares) is pulled out by one identity-masked
    scalar_tensor_tensor with accum_out per bank.  Effective rate
    ~0.63 ns per column-of-all-rows -- faster than ACT's 0.90.
DMAs are interleaved (xt group / nat tile) in consumption order with
bufs-bounded pools so all three engines stream without startup stalls.
fp8 quantization only perturbs the NORM (the host computes e_y from
full-precision x): S2 rel err ~0.1% -> loss rel err ~1e-7 (validated).
Exact f64 fallback for any row with nonfinite/nonpositive S2.
"""

import numpy as np

_BATCH = 8192
_N = 8192
_NCORES = 8
_RPC = _BATCH // _NCORES  # rows per core
_P = 128
_TILES = _RPC // _P  # row-tiles (and PE row-blocks) per core
_K = 5.0
_TAU = 1.0
_EPS = 1.0

_NC_CACHE = {}
VARIANT = "hyb8"

# (act_cols, dve_cols, pe_chunks): column split per 8192-wide row set.
# HW rates: ACT (A+352)/1.2 ns + 186/tile, DVE-STT (D+151)/0.96 ns,
# PE ~56 ns warm per (128-col chunk, 128-row block) LDW+MM pair.
_SPLITS = {
    "hyb": (2432, 2176, 28),
    "hyb2": (2816, 2048, 26),
    "hyb3": (2304, 1920, 31),
    "hyb4": (2304, 1920, 31),
    "hyb4g": (2304, 1920, 31),  # xt stream via gpsimd SWDGE ring
    "hyb5": (2304, 1920, 31),
    "hyb6": (2304, 1920, 31),
    "hyb6g": (2304, 1920, 31),  # probe: all DMAs on the sync ring
    "hyb7": (2304, 1920, 31),
    "hyb8": (2176, 1920, 32),
    "hyb8b": (2176, 1920, 32),
    "sq8": (4480, 3712, 0),  # fallback: no PE (old baseline split)
}


def _params(variant):
    a_cols, d_cols, pe_chunks = _SPLITS[variant]
    assert a_cols + d_cols + pe_chunks * _P == _N
    return a_cols, d_cols, pe_chunks


def _build_bass(variant=None):
    import concourse.bacc as bacc
    import concourse.mybir as mybir
    import concourse.tile as tile

    if variant is None:
        variant = VARIANT
    a_cols, d_cols, pe_chunks = _params(variant)
    nat_cols = a_cols + d_cols
    # xt groups of 4 chunks (one DMA each)
    GRP = 4
    n_grp = (pe_chunks + GRP - 1) // GRP

    nc = bacc.Bacc(
        "TRN2", target_bir_lowering=False, debug=False, num_devices=_NCORES
    )
    f32 = mybir.dt.float32
    bf16 = mybir.dt.bfloat16
    f8 = mybir.dt.float8e4
    AF = mybir.ActivationFunctionType
    ALU = mybir.AluOpType

    xn = nc.dram_tensor("xn", [_RPC, nat_cols], f8, kind="ExternalInput")
    if pe_chunks:
        # packed transposed chunks: xt[p, c*RPC + r] = x[r, nat+c*128+p]
        xt = nc.dram_tensor(
            "xt", [_P, pe_chunks * _RPC], f8, kind="ExternalInput"
        )
        ident = nc.dram_tensor("ident", [_P, _P], bf16, kind="ExternalInput")
    k_st = 3 if pe_chunks else 2
    stats = nc.dram_tensor("stats", [_P, k_st * _TILES], f32, kind="ExternalOutput")

    with tile.TileContext(nc) as tc:
        with (
            tc.tile_pool(name="xnp", bufs=3) as xnp,
            tc.tile_pool(name="xtp", bufs=3) as xtp,
            tc.tile_pool(name="sp", bufs=2) as sp,
            tc.tile_pool(name="singles", bufs=1) as singles,
            tc.tile_pool(name="psum", bufs=1, space="PSUM") as psp,
        ):
            stA = singles.tile([_P, _TILES], f32, name="stA")
            stD = singles.tile([_P, _TILES], f32, name="stD")
            nc.vector.memset(stA, 0.0)
            nc.vector.memset(stD, 0.0)
            if pe_chunks:
                stG = singles.tile([_P, _TILES], f32, name="stG")
                nc.vector.memset(stG, 0.0)
                id_t = singles.tile([_P, _P], bf16, name="id_t")
                nc.sync.dma_start(out=id_t, in_=ident[:, :])
                gram = [
                    psp.tile([_P, 512], f32, name=f"gram_{rb}")
                    for rb in range(_TILES)
                ]
                # Dummy 1-elem Square hoists the ACT table load so it
                # overlaps the head DMAs instead of the first real op.
                warm = singles.tile([_P, 1], f32, name="warm")
                nc.scalar.activation(warm, id_t[:, 0:1], AF.Square)
            else:
                warm = singles.tile([_P, 1], f32, name="warm")
                nc.scalar.activation(warm, stA[:, 0:1], AF.Square)

            def nat_tile(t):
                x_tile = xnp.tile([_P, nat_cols], f8, tag="xn", name=f"xn_{t}")
                nc.sync.dma_start(out=x_tile, in_=xn[t * _P : (t + 1) * _P, :])
                scrA = sp.tile([_P, a_cols], f8, tag="scrA", name=f"sa_{t}")
                nc.scalar.activation(
                    scrA,
                    x_tile[:, :a_cols],
                    AF.Square,
                    accum_out=stA[:, t : t + 1],
                )
                scrD = sp.tile([_P, d_cols], f8, tag="scrD", name=f"sd_{t}")
                nc.vector.scalar_tensor_tensor(
                    out=scrD,
                    in0=x_tile[:, a_cols:],
                    scalar=1.0,
                    in1=x_tile[:, a_cols:],
                    op0=ALU.mult,
                    op1=ALU.mult,
                    accum_out=stD[:, t : t + 1],
                )

            def xt_group(g):
                lo = g * GRP
                hi = min(lo + GRP, pe_chunks)
                w = (hi - lo) * _RPC
                xt_t = xtp.tile([_P, w], f8, tag="xt", name=f"xt_{g}")
                nc.sync.dma_start(
                    out=xt_t, in_=xt[:, lo * _RPC : lo * _RPC + w]
                )
                for l in range(hi - lo):
                    c = lo + l
                    for rb in range(_TILES):
                        off = l * _RPC + rb * _P
                        sl = xt_t[:, off : off + _P]
                        nc.tensor.matmul(
                            gram[rb][:, :_P],
                            sl,
                            sl,
                            start=(c == 0),
                            stop=(c == pe_chunks - 1),
                        )

            # Interleave DMA/compute issue in consumption order: PE first
            # (it can start before ACT's table load finishes), then
            # alternate nat tiles and xt groups.
            if pe_chunks:
                xt_group(0)
            nat_tile(0)
            for i in range(1, max(n_grp, _TILES)):
                if pe_chunks and i < n_grp:
                    xt_group(i)
                if i < _TILES:
                    nat_tile(i)

            if pe_chunks:
                # diag(gram[rb]) via identity-masked STT, fused accum.
                for rb in range(_TILES):
                    dscr = sp.tile([_P, _P], bf16, tag="dscr", name=f"dg_{rb}")
                    nc.vector.scalar_tensor_tensor(
                        out=dscr,
                        in0=gram[rb][:, :_P],
                        scalar=1.0,
                        in1=id_t,
                        op0=ALU.mult,
                        op1=ALU.mult,
                        accum_out=stG[:, rb : rb + 1],
                    )

            nc.sync.dma_start(out=stats[:, 0:_TILES], in_=stA)
            nc.sync.dma_start(out=stats[:, _TILES : 2 * _TILES], in_=stD)
            if pe_chunks:
                nc.sync.dma_start(out=stats[:, 2 * _TILES :], in_=stG)
    nc.finalize()
    return nc


def _build_hyb2(variant="hyb2"):
    """v2: express-lane head DMAs on the ACT HWDGE ring, deep-buffered
    bulk stream on the SP ring, last tile split into sub-chunks with
    separate accumulator sections, per-engine contiguous stats DMAs."""
    import concourse.bacc as bacc
    import concourse.mybir as mybir
    import concourse.tile as tile

    a_cols, d_cols, pe_chunks = _params(variant)
    nat_cols = a_cols + d_cols
    n_grp = (pe_chunks + 3) // 4  # 4-chunk groups; last may be short

    nc = bacc.Bacc(
        "TRN2", target_bir_lowering=False, debug=False, num_devices=_NCORES
    )
    f32 = mybir.dt.float32
    bf16 = mybir.dt.bfloat16
    f8 = mybir.dt.float8e4
    AF = mybir.ActivationFunctionType
    ALU = mybir.AluOpType

    xn = nc.dram_tensor("xn", [_RPC, nat_cols], f8, kind="ExternalInput")
    xt = nc.dram_tensor("xt", [_P, pe_chunks * _RPC], f8, kind="ExternalInput")
    ident = nc.dram_tensor("ident", [_P, _P], bf16, kind="ExternalInput")
    # cols 0-7 ACT tiles (7 = sub-op a), 8 ACT t7 sub-op b,
    # 9-16 DVE tiles (16 = sub-op a), 17 DVE t7 sub-op b, 18-25 PE blocks
    NSEC = 26
    stats = nc.dram_tensor("stats", [_P, NSEC], f32, kind="ExternalOutput")

    # t7 sub-chunk boundaries (within nat cols)
    a_mid = 2048
    d_mid = a_cols + ((d_cols * 3) // 4 // _P) * _P  # last DVE chunk small

    with tile.TileContext(nc) as tc:
        with (
            tc.tile_pool(name="xnp", bufs=4) as xnp,
            tc.tile_pool(name="xtp", bufs=3) as xtp,
            tc.tile_pool(name="sp", bufs=2) as sp,
            tc.tile_pool(name="singles", bufs=1) as singles,
            tc.tile_pool(name="psum", bufs=1, space="PSUM") as psp,
        ):
            st = singles.tile([_P, NSEC], f32, name="st")
            nc.vector.memset(st, 0.0)
            id_t = singles.tile([_P, _P], bf16, name="id_t")
            gram = [
                psp.tile([_P, 512], f32, name=f"gram_{rb}")
                for rb in range(_TILES)
            ]

            nat = [
                xnp.tile([_P, nat_cols], f8, tag="xn", name=f"xn_{t}")
                for t in range(_TILES)
            ]
            xtt = []
            for g in range(n_grp):
                w = (min((g + 1) * 4, pe_chunks) - g * 4) * _RPC
                xtt.append(xtp.tile([_P, w], f8, tag="xt", name=f"xt_{g}"))

            # ── express head (ACT HWDGE ring): ident, xt chunks 0-3,
            # nat tiles 0-1 split per engine section ──
            nc.scalar.dma_start(out=id_t, in_=ident[:, :])
            nc.scalar.dma_start(
                out=xtt[0][:, :_RPC], in_=xt[:, 0:_RPC]
            )
            for t in (0, 1):
                nc.scalar.dma_start(
                    out=nat[t][:, :a_cols],
                    in_=xn[t * _P : (t + 1) * _P, :a_cols],
                )
                nc.scalar.dma_start(
                    out=nat[t][:, a_cols:],
                    in_=xn[t * _P : (t + 1) * _P, a_cols:],
                )
                if t == 0:
                    for c in (1, 2, 3):
                        nc.scalar.dma_start(
                            out=xtt[0][:, c * _RPC : (c + 1) * _RPC],
                            in_=xt[:, c * _RPC : (c + 1) * _RPC],
                        )

            # ── bulk stream (SP ring), xt-leading interleave ──
            for i in range(1, max(n_grp, _TILES - 1)):
                if i < n_grp:
                    lo = i * 4 * _RPC
                    nc.sync.dma_start(
                        out=xtt[i], in_=xt[:, lo : lo + xtt[i].shape[1]]
                    )
                t = i + 1
                if 2 <= t < _TILES - 1:
                    nc.sync.dma_start(
                        out=nat[t], in_=xn[t * _P : (t + 1) * _P, :]
                    )
            # last tile in 4 sub-chunks (small final arrivals)
            t = _TILES - 1
            r0 = t * _P
            for lo, hi in (
                (0, a_mid),
                (a_mid, a_cols),
                (a_cols, d_mid),
                (d_mid, nat_cols),
            ):
                nc.sync.dma_start(
                    out=nat[t][:, lo:hi], in_=xn[r0 : r0 + _P, lo:hi]
                )

            # ── ScalarE: table-load warm + squares ──
            warm = singles.tile([_P, 1], f32, name="warm")
            nc.scalar.activation(warm, id_t[:, 0:1], AF.Square)
            for t in range(_TILES):
                if t < _TILES - 1:
                    scrA = sp.tile([_P, a_cols], f8, tag="scrA", name=f"sa_{t}")
                    nc.scalar.activation(
                        scrA,
                        nat[t][:, :a_cols],
                        AF.Square,
                        accum_out=st[:, t : t + 1],
                    )
                else:
                    scrA = sp.tile([_P, a_cols], f8, tag="scrA", name=f"sa_{t}")
                    nc.scalar.activation(
                        scrA[:, :a_mid],
                        nat[t][:, :a_mid],
                        AF.Square,
                        accum_out=st[:, 7:8],
                    )
                    nc.scalar.activation(
                        scrA[:, a_mid:],
                        nat[t][:, a_mid:a_cols],
                        AF.Square,
                        accum_out=st[:, 8:9],
                    )
            nc.scalar.dma_start(out=stats[:, 0:9], in_=st[:, 0:9])

            # ── TensorE: Gram-block accumulation, chunk-major ──
            for c in range(pe_chunks):
                g, l = c // 4, c % 4
                for rb in range(_TILES):
                    off = l * _RPC + rb * _P
                    sl = xtt[g][:, off : off + _P]
                    nc.tensor.matmul(
                        gram[rb][:, :_P],
                        sl,
                        sl,
                        start=(c == 0),
                        stop=(c == pe_chunks - 1),
                    )

            # ── VectorE: STT squares; diags fill the last-tile DMA wait ──
            def stt(t, lo, hi, sec):
                scrD = sp.tile(
                    [_P, hi - lo], f8, tag="scrD", name=f"sd_{sec}"
                )
                nc.vector.scalar_tensor_tensor(
                    out=scrD,
                    in0=nat[t][:, lo:hi],
                    scalar=1.0,
                    in1=nat[t][:, lo:hi],
                    op0=ALU.mult,
                    op1=ALU.mult,
                    accum_out=st[:, sec : sec + 1],
                )

            for t in range(_TILES - 1):
                stt(t, a_cols, nat_cols, 9 + t)
            for rb in range(_TILES):
                dscr = sp.tile([_P, _P], bf16, tag="dscr", name=f"dg_{rb}")
                nc.vector.scalar_tensor_tensor(
                    out=dscr,
                    in0=gram[rb][:, :_P],
                    scalar=1.0,
                    in1=id_t,
                    op0=ALU.mult,
                    op1=ALU.mult,
                    accum_out=st[:, 18 + rb : 19 + rb],
                )
            nc.sync.dma_start(out=stats[:, 18:26], in_=st[:, 18:26])
            stt(_TILES - 1, a_cols, d_mid, 16)
            stt(_TILES - 1, d_mid, nat_cols, 17)
            nc.sync.dma_start(out=stats[:, 9:18], in_=st[:, 9:18])
    nc.finalize()
    return nc


def _build_hyb3(variant="hyb3"):
    """v3: everything resident in SBUF (no pool recycling) so every DMA
    is issued up front and the stream runs at the HBM roofline start to
    finish; first-needed chunks issued first in small pieces so compute
    starts early; last tile in small sub-chunks to shrink the tail."""
    import concourse.bacc as bacc
    import concourse.mybir as mybir
    import concourse.tile as tile

    a_cols, d_cols, pe_chunks = _params(variant)
    nat_cols = a_cols + d_cols
    n_grp = (pe_chunks + 3) // 4

    nc = bacc.Bacc(
        "TRN2", target_bir_lowering=False, debug=False, num_devices=_NCORES
    )
    f32 = mybir.dt.float32
    bf16 = mybir.dt.bfloat16
    f8 = mybir.dt.float8e4
    AF = mybir.ActivationFunctionType
    ALU = mybir.AluOpType

    xn = nc.dram_tensor("xn", [_RPC, nat_cols], f8, kind="ExternalInput")
    xt = nc.dram_tensor("xt", [_P, pe_chunks * _RPC], f8, kind="ExternalInput")
    ident = nc.dram_tensor("ident", [_P, _P], bf16, kind="ExternalInput")
    NSEC = 26
    stats = nc.dram_tensor("stats", [_P, NSEC], f32, kind="ExternalOutput")

    a_mid = (a_cols * 3 // 4 // _P) * _P  # t7 ACT sub-split (small 2nd op)
    d_mid = a_cols + (d_cols * 3 // 4 // _P) * _P

    with tile.TileContext(nc) as tc:
        with (
            tc.tile_pool(name="res", bufs=1) as res,
            tc.tile_pool(name="sp", bufs=2) as sp,
            tc.tile_pool(name="psum", bufs=1, space="PSUM") as psp,
        ):
            st = res.tile([_P, NSEC], f32, name="st")
            nc.vector.memset(st, 0.0)
            id_t = res.tile([_P, _P], bf16, name="id_t")
            gram = [
                psp.tile([_P, 512], f32, name=f"gram_{rb}")
                for rb in range(_TILES)
            ]
            nat = [
                res.tile([_P, nat_cols], f8, name=f"xn_{t}")
                for t in range(_TILES)
            ]
            xtt = [
                res.tile(
                    [_P, (min((g + 1) * 4, pe_chunks) - g * 4) * _RPC],
                    f8,
                    name=f"xt_{g}",
                )
                for g in range(n_grp)
            ]

            def nat_dma(t, lo, hi):
                nc.sync.dma_start(
                    out=nat[t][:, lo:hi],
                    in_=xn[t * _P : (t + 1) * _P, lo:hi],
                )

            # head: tile-0 engine sections in halves, then xt chunk 0-3,
            # then ident (needed ~7us for table-load warm)
            nat_dma(0, 0, a_cols // 2)
            nat_dma(0, a_cols // 2, a_cols)
            nat_dma(0, a_cols, a_cols + d_cols // 2)
            nat_dma(0, a_cols + d_cols // 2, nat_cols)
            for c in range(4):
                nc.sync.dma_start(
                    out=xtt[0][:, c * _RPC : (c + 1) * _RPC],
                    in_=xt[:, c * _RPC : (c + 1) * _RPC],
                )
            nc.sync.dma_start(out=id_t, in_=ident[:, :])
            nat_dma(1, 0, a_cols)
            nat_dma(1, a_cols, nat_cols)
            # bulk: xt-leading alternation, whole transfers
            for i in range(1, max(n_grp, _TILES - 1)):
                if i < n_grp:
                    lo = i * 4 * _RPC
                    nc.sync.dma_start(
                        out=xtt[i], in_=xt[:, lo : lo + xtt[i].shape[1]]
                    )
                t = i + 1
                if 2 <= t < _TILES - 1:
                    nat_dma(t, 0, nat_cols)
            # tail: last tile in 4 sub-chunks, smallest last
            t = _TILES - 1
            nat_dma(t, 0, a_mid)
            nat_dma(t, a_mid, a_cols)
            nat_dma(t, a_cols, d_mid)
            nat_dma(t, d_mid, nat_cols)

            # ── ScalarE ──
            warm = res.tile([_P, 1], f32, name="warm")
            nc.scalar.activation(warm, id_t[:, 0:1], AF.Square)
            for t in range(_TILES):
                scrA = sp.tile([_P, a_cols], f8, tag="scrA", name=f"sa_{t}")
                if t < _TILES - 1:
                    nc.scalar.activation(
                        scrA,
                        nat[t][:, :a_cols],
                        AF.Square,
                        accum_out=st[:, t : t + 1],
                    )
                else:
                    nc.scalar.activation(
                        scrA[:, :a_mid],
                        nat[t][:, :a_mid],
                        AF.Square,
                        accum_out=st[:, 7:8],
                    )
                    nc.scalar.activation(
                        scrA[:, a_mid:],
                        nat[t][:, a_mid:a_cols],
                        AF.Square,
                        accum_out=st[:, 8:9],
                    )

            # ── TensorE ──
            for c in range(pe_chunks):
                g, l = c // 4, c % 4
                for rb in range(_TILES):
                    off = l * _RPC + rb * _P
                    sl = xtt[g][:, off : off + _P]
                    nc.tensor.matmul(
                        gram[rb][:, :_P],
                        sl,
                        sl,
                        start=(c == 0),
                        stop=(c == pe_chunks - 1),
                    )

            # ── VectorE ──
            def stt(t, lo, hi, sec):
                scrD = sp.tile([_P, hi - lo], f8, tag="scrD", name=f"sd_{sec}")
                nc.vector.scalar_tensor_tensor(
                    out=scrD,
                    in0=nat[t][:, lo:hi],
                    scalar=1.0,
                    in1=nat[t][:, lo:hi],
                    op0=ALU.mult,
                    op1=ALU.mult,
                    accum_out=st[:, sec : sec + 1],
                )

            for t in range(_TILES - 1):
                stt(t, a_cols, nat_cols, 9 + t)
            for rb in range(_TILES):
                dscr = sp.tile([_P, _P], bf16, tag="dscr", name=f"dg_{rb}")
                nc.vector.scalar_tensor_tensor(
                    out=dscr,
                    in0=gram[rb][:, :_P],
                    scalar=1.0,
                    in1=id_t,
                    op0=ALU.mult,
                    op1=ALU.mult,
                    accum_out=st[:, 18 + rb : 19 + rb],
                )
            nc.sync.dma_start(out=stats[:, 18:26], in_=st[:, 18:26])
            nc.scalar.dma_start(out=stats[:, 0:9], in_=st[:, 0:9])
            stt(_TILES - 1, a_cols, d_mid, 16)
            stt(_TILES - 1, d_mid, nat_cols, 17)
            nc.sync.dma_start(out=stats[:, 9:18], in_=st[:, 9:18])
    nc.finalize()
    return nc


def _build_hyb4(variant="hyb4"):
    """v4: all tiles resident (DMA fully decoupled from compute), large
    ~1MB transfers for descriptor efficiency, diag extraction placed to
    overlap the last nat tile's DMA wait, small final sub-chunks."""
    import concourse.bacc as bacc
    import concourse.mybir as mybir
    import concourse.tile as tile

    a_cols, d_cols, pe_chunks = _params(variant)
    nat_cols = a_cols + d_cols
    xt_dma = nc_dma = None  # set below

    nc = bacc.Bacc(
        "TRN2", target_bir_lowering=False, debug=False, num_devices=_NCORES
    )
    f32 = mybir.dt.float32
    bf16 = mybir.dt.bfloat16
    f8 = mybir.dt.float8e4
    AF = mybir.ActivationFunctionType
    ALU = mybir.AluOpType

    xt_engine = nc.gpsimd if variant.endswith("g") else nc.sync

    xn = nc.dram_tensor("xn", [_RPC, nat_cols], f8, kind="ExternalInput")
    xt = nc.dram_tensor("xt", [_P, pe_chunks * _RPC], f8, kind="ExternalInput")
    ident = nc.dram_tensor("ident", [_P, _P], bf16, kind="ExternalInput")
    NSEC = 26
    stats = nc.dram_tensor("stats", [_P, NSEC], f32, kind="ExternalOutput")

    a_mid = (a_cols * 3 // 4 // _P) * _P
    d_mid = a_cols + (d_cols * 3 // 4 // _P) * _P

    # xt transfer groups: 8 chunks (1 MB) each, last group short
    GRP = 8
    n_grp = (pe_chunks + GRP - 1) // GRP

    with tile.TileContext(nc) as tc:
        with (
            tc.tile_pool(name="res", bufs=1) as res,
            tc.tile_pool(name="sp", bufs=2) as sp,
            tc.tile_pool(name="psum", bufs=1, space="PSUM") as psp,
        ):
            st = res.tile([_P, NSEC], f32, name="st")
            nc.vector.memset(st, 0.0)
            id_t = res.tile([_P, _P], bf16, name="id_t")
            gram = [
                psp.tile([_P, 512], f32, name=f"gram_{rb}")
                for rb in range(_TILES)
            ]
            nat = [
                res.tile([_P, nat_cols], f8, name=f"xn_{t}")
                for t in range(_TILES)
            ]
            xtt = [
                res.tile(
                    [_P, (min((g + 1) * GRP, pe_chunks) - g * GRP) * _RPC],
                    f8,
                    name=f"xt_{g}",
                )
                for g in range(n_grp)
            ]

            def nat_dma(t, lo, hi):
                nc.sync.dma_start(
                    out=nat[t][:, lo:hi],
                    in_=xn[t * _P : (t + 1) * _P, lo:hi],
                )

            def xt_dma(g):
                lo = g * GRP * _RPC
                xt_engine.dma_start(
                    out=xtt[g], in_=xt[:, lo : lo + xtt[g].shape[1]]
                )

            # DMA issue order = rough consumption order; queue-FIFO
            # stacking keeps the stream saturated to the end.
            nc.sync.dma_start(out=id_t, in_=ident[:, :])
            nat_dma(0, 0, nat_cols)
            xt_dma(0)
            nat_dma(1, 0, nat_cols)
            xt_dma(1)
            nat_dma(2, 0, nat_cols)
            xt_dma(2)
            nat_dma(3, 0, nat_cols)
            if n_grp > 3:
                xt_dma(3)
            for t in range(4, _TILES - 1):
                nat_dma(t, 0, nat_cols)
            t = _TILES - 1
            nat_dma(t, 0, a_mid)
            nat_dma(t, a_mid, a_cols)
            nat_dma(t, a_cols, d_mid)
            nat_dma(t, d_mid, nat_cols)

            # ── ScalarE ──
            warm = res.tile([_P, 1], f32, name="warm")
            nc.scalar.activation(warm, st[:, 0:1], AF.Square)
            for t in range(_TILES):
                scrA = sp.tile([_P, a_cols], f8, tag="scrA", name=f"sa_{t}")
                if t < _TILES - 1:
                    nc.scalar.activation(
                        scrA,
                        nat[t][:, :a_cols],
                        AF.Square,
                        accum_out=st[:, t : t + 1],
                    )
                else:
                    nc.scalar.activation(
                        scrA[:, :a_mid],
                        nat[t][:, :a_mid],
                        AF.Square,
                        accum_out=st[:, 7:8],
                    )
                    nc.scalar.activation(
                        scrA[:, a_mid:],
                        nat[t][:, a_mid:a_cols],
                        AF.Square,
                        accum_out=st[:, 8:9],
                    )

            # ── TensorE ──
            for c in range(pe_chunks):
                g, l = c // GRP, c % GRP
                for rb in range(_TILES):
                    off = l * _RPC + rb * _P
                    sl = xtt[g][:, off : off + _P]
                    nc.tensor.matmul(
                        gram[rb][:, :_P],
                        sl,
                        sl,
                        start=(c == 0),
                        stop=(c == pe_chunks - 1),
                    )

            # ── VectorE ──
            def stt(t, lo, hi, sec):
                scrD = sp.tile([_P, hi - lo], f8, tag="scrD", name=f"sd_{sec}")
                nc.vector.scalar_tensor_tensor(
                    out=scrD,
                    in0=nat[t][:, lo:hi],
                    scalar=1.0,
                    in1=nat[t][:, lo:hi],
                    op0=ALU.mult,
                    op1=ALU.mult,
                    accum_out=st[:, sec : sec + 1],
                )

            for t in range(_TILES - 1):
                stt(t, a_cols, nat_cols, 9 + t)
            for rb in range(_TILES):
                dscr = sp.tile([_P, _P], bf16, tag="dscr", name=f"dg_{rb}")
                nc.vector.scalar_tensor_tensor(
                    out=dscr,
                    in0=gram[rb][:, :_P],
                    scalar=1.0,
                    in1=id_t,
                    op0=ALU.mult,
                    op1=ALU.mult,
                    accum_out=st[:, 18 + rb : 19 + rb],
                )
            nc.sync.dma_start(out=stats[:, 18:26], in_=st[:, 18:26])
            stt(_TILES - 1, a_cols, d_mid, 16)
            stt(_TILES - 1, d_mid, nat_cols, 17)
            nc.scalar.dma_start(out=stats[:, 0:9], in_=st[:, 0:9])
            nc.sync.dma_start(out=stats[:, 9:18], in_=st[:, 9:18])
    nc.finalize()
    return nc


def _build_hyb5(variant="hyb5"):
    """v5: dma_start issue costs ~650ns serialized per sequencer and the
    HWDGE ring caps ~10 in-flight transfers, so use FEW large DMAs and
    split them across two independent issue paths: sync HWDGE carries
    the host-packed natural stream, gpsimd SWDGE carries the transposed
    PE stream.  All tiles resident; compute slices one big nat tile."""
    import concourse.bacc as bacc
    import concourse.mybir as mybir
    import concourse.tile as tile

    a_cols, d_cols, pe_chunks = _params(variant)
    nat_cols = a_cols + d_cols

    nc = bacc.Bacc(
        "TRN2", target_bir_lowering=False, debug=False, num_devices=_NCORES
    )
    f32 = mybir.dt.float32
    bf16 = mybir.dt.bfloat16
    f8 = mybir.dt.float8e4
    AF = mybir.ActivationFunctionType
    ALU = mybir.AluOpType

    # xn2[p, t*nat+c] = x[t*128+p, c]  (host-packed row-tile-major)
    xn = nc.dram_tensor(
        "xn2", [_P, _TILES * nat_cols], f8, kind="ExternalInput"
    )
    xt = nc.dram_tensor("xt", [_P, pe_chunks * _RPC], f8, kind="ExternalInput")
    ident = nc.dram_tensor("ident", [_P, _P], bf16, kind="ExternalInput")
    NSEC = 26
    stats = nc.dram_tensor("stats", [_P, NSEC], f32, kind="ExternalOutput")

    a_mid = (a_cols * 3 // 4 // _P) * _P
    d_mid = a_cols + (d_cols * 3 // 4 // _P) * _P

    GRP = 8
    n_grp = (pe_chunks + GRP - 1) // GRP

    with tile.TileContext(nc) as tc:
        with (
            tc.tile_pool(name="res", bufs=1) as res,
            tc.tile_pool(name="sp", bufs=2) as sp,
            tc.tile_pool(name="psum", bufs=1, space="PSUM") as psp,
        ):
            st = res.tile([_P, NSEC], f32, name="st")
            nc.vector.memset(st, 0.0)
            id_t = res.tile([_P, _P], bf16, name="id_t")
            gram = [
                psp.tile([_P, 512], f32, name=f"gram_{rb}")
                for rb in range(_TILES)
            ]
            xna = res.tile([_P, _TILES * nat_cols], f8, name="xna")
            xtt = [
                res.tile(
                    [_P, (min((g + 1) * GRP, pe_chunks) - g * GRP) * _RPC],
                    f8,
                    name=f"xt_{g}",
                )
                for g in range(n_grp)
            ]

            def nat_ap(t, lo, hi):
                return xna[:, t * nat_cols + lo : t * nat_cols + hi]

            def nat_dma(eng, lo_t, lo, hi_t, hi):
                a, b = lo_t * nat_cols + lo, hi_t * nat_cols + hi
                eng.dma_start(out=xna[:, a:b], in_=xn[:, a:b])

            def xt_dma(eng, g):
                lo = g * GRP * _RPC
                eng.dma_start(out=xtt[g], in_=xt[:, lo : lo + xtt[g].shape[1]])

            t7 = _TILES - 1
            if variant.endswith("g"):
                # probe: everything on the sync ring
                nc.sync.dma_start(out=id_t, in_=ident[:, :])
                for t in range(_TILES - 1):
                    nat_dma(nc.sync, t, 0, t, nat_cols)
                for g in range(n_grp):
                    xt_dma(nc.sync, g)
                nat_dma(nc.sync, t7, 0, t7, a_mid)
                nat_dma(nc.sync, t7, a_mid, t7, d_mid)
                nat_dma(nc.sync, t7, d_mid, t7, nat_cols)
            elif variant.startswith("hyb7"):
                # scalar ring kept short (3 issues) so ACT's queue frees
                # early; xt front-loaded so PE + diags finish mid-kernel.
                nc.scalar.dma_start(out=id_t, in_=ident[:, :])
                xt_dma(nc.scalar, 1)
                xt_dma(nc.scalar, 2)
                nat_dma(nc.sync, 0, 0, 0, nat_cols)
                nat_dma(nc.sync, 1, 0, 1, nat_cols)
                xt_dma(nc.sync, 0)
                nat_dma(nc.sync, 2, 0, 2, nat_cols)
                xt_dma(nc.sync, 3)
                nat_dma(nc.sync, 3, 0, 3, nat_cols)
                nat_dma(nc.sync, 4, 0, 4, nat_cols)
                nat_dma(nc.sync, 5, 0, 5, nat_cols)
                nat_dma(nc.sync, 6, 0, 6, nat_cols)
                nat_dma(nc.sync, t7, 0, t7, a_mid)
                nat_dma(nc.sync, t7, a_mid, t7, d_mid)
                nat_dma(nc.sync, t7, d_mid, t7, nat_cols)
            else:
                # two HWDGE rings in parallel: each ring FIFO, both
                # drive all 16 SDMA engines; alternate streams so
                # arrival order tracks consumption order.
                nc.scalar.dma_start(out=id_t, in_=ident[:, :])
                nat_dma(nc.sync, 0, 0, 0, nat_cols)
                nat_dma(nc.scalar, 1, 0, 1, nat_cols)
                xt_dma(nc.sync, 0)
                nat_dma(nc.scalar, 2, 0, 2, nat_cols)
                nat_dma(nc.sync, 3, 0, 3, nat_cols)
                xt_dma(nc.scalar, 1)
                nat_dma(nc.sync, 4, 0, 4, nat_cols)
                xt_dma(nc.scalar, 2)
                nat_dma(nc.sync, 5, 0, 5, nat_cols)
                xt_dma(nc.scalar, 3)
                nat_dma(nc.sync, 6, 0, 6, nat_cols)
                nat_dma(nc.sync, t7, 0, t7, a_mid)
                nat_dma(nc.sync, t7, a_mid, t7, d_mid)
                nat_dma(nc.sync, t7, d_mid, t7, nat_cols)

            # ── ScalarE ──
            warm = res.tile([_P, 1], f32, name="warm")
            nc.scalar.activation(warm, st[:, 0:1], AF.Square)
            for t in range(_TILES):
                scrA = sp.tile([_P, a_cols], f8, tag="scrA", name=f"sa_{t}")
                if t < _TILES - 1:
                    nc.scalar.activation(
                        scrA,
                        nat_ap(t, 0, a_cols),
                        AF.Square,
                        accum_out=st[:, t : t + 1],
                    )
                else:
                    nc.scalar.activation(
                        scrA[:, :a_mid],
                        nat_ap(t, 0, a_mid),
                        AF.Square,
                        accum_out=st[:, 7:8],
                    )
                    nc.scalar.activation(
                        scrA[:, a_mid:],
                        nat_ap(t, a_mid, a_cols),
                        AF.Square,
                        accum_out=st[:, 8:9],
                    )

            # ── TensorE ──
            for c in range(pe_chunks):
                g, l = c // GRP, c % GRP
                for rb in range(_TILES):
                    off = l * _RPC + rb * _P
                    sl = xtt[g][:, off : off + _P]
                    nc.tensor.matmul(
                        gram[rb][:, :_P],
                        sl,
                        sl,
                        start=(c == 0),
                        stop=(c == pe_chunks - 1),
                    )

            # ── VectorE ──
            def stt(t, lo, hi, sec):
                scrD = sp.tile([_P, hi - lo], f8, tag="scrD", name=f"sd_{sec}")
                nc.vector.scalar_tensor_tensor(
                    out=scrD,
                    in0=nat_ap(t, lo, hi),
                    scalar=1.0,
                    in1=nat_ap(t, lo, hi),
                    op0=ALU.mult,
                    op1=ALU.mult,
                    accum_out=st[:, sec : sec + 1],
                )

            for t in range(_TILES - 1):
                stt(t, a_cols, nat_cols, 9 + t)
            for rb in range(_TILES):
                dscr = sp.tile([_P, _P], bf16, tag="dscr", name=f"dg_{rb}")
                nc.vector.scalar_tensor_tensor(
                    out=dscr,
                    in0=gram[rb][:, :_P],
                    scalar=1.0,
                    in1=id_t,
                    op0=ALU.mult,
                    op1=ALU.mult,
                    accum_out=st[:, 18 + rb : 19 + rb],
                )
            nc.sync.dma_start(out=stats[:, 18:26], in_=st[:, 18:26])
            stt(_TILES - 1, a_cols, d_mid, 16)
            stt(_TILES - 1, d_mid, nat_cols, 17)
            nc.scalar.dma_start(out=stats[:, 0:9], in_=st[:, 0:9])
            nc.sync.dma_start(out=stats[:, 9:18], in_=st[:, 9:18])
    nc.finalize()
    return nc


def _build_hyb8(variant="hyb8"):
    """v8: two-ring issue (scalar-ring dma_starts interleaved between
    ACT's squares so ACT starts early), PE consumes xt groups in
    arrival order, balanced ring bytes, small final sub-chunks."""
    import concourse.bacc as bacc
    import concourse.mybir as mybir
    import concourse.tile as tile

    a_cols, d_cols, pe_chunks = _params(variant)
    nat_cols = a_cols + d_cols

    nc = bacc.Bacc(
        "TRN2", target_bir_lowering=False, debug=False, num_devices=_NCORES
    )
    f32 = mybir.dt.float32
    bf16 = mybir.dt.bfloat16
    f8 = mybir.dt.float8e4
    AF = mybir.ActivationFunctionType
    ALU = mybir.AluOpType

    xn = nc.dram_tensor(
        "xn2", [_P, _TILES * nat_cols], f8, kind="ExternalInput"
    )
    xt = nc.dram_tensor("xt", [_P, pe_chunks * _RPC], f8, kind="ExternalInput")
    ident = nc.dram_tensor("ident", [_P, _P], bf16, kind="ExternalInput")
    NSEC = 26
    stats = nc.dram_tensor("stats", [_P, NSEC], f32, kind="ExternalOutput")

    a_mid = (a_cols * 3 // 4 // _P) * _P
    d_mid = a_cols + (d_cols * 3 // 4 // _P) * _P

    GRP = 8
    n_grp = (pe_chunks + GRP - 1) // GRP

    with tile.TileContext(nc) as tc:
        with (
            tc.tile_pool(name="res", bufs=1) as res,
            tc.tile_pool(name="sp", bufs=2) as sp,
            tc.tile_pool(name="psum", bufs=1, space="PSUM") as psp,
        ):
            st = res.tile([_P, NSEC], f32, name="st")
            nc.vector.memset(st, 0.0)
            id_t = res.tile([_P, _P], bf16, name="id_t")
            gram = [
                psp.tile([_P, 512], f32, name=f"gram_{rb}")
                for rb in range(_TILES)
            ]
            xna = res.tile([_P, _TILES * nat_cols], f8, name="xna")
            xtt = [
                res.tile(
                    [_P, (min((g + 1) * GRP, pe_chunks) - g * GRP) * _RPC],
                    f8,
                    name=f"xt_{g}",
                )
                for g in range(n_grp)
            ]

            def nat_ap(t, lo, hi):
                return xna[:, t * nat_cols + lo : t * nat_cols + hi]

            def nat_dma(eng, lo_t, lo, hi_t, hi):
                a, b = lo_t * nat_cols + lo, hi_t * nat_cols + hi
                eng.dma_start(out=xna[:, a:b], in_=xn[:, a:b])

            def xt_dma(eng, g):
                lo = g * GRP * _RPC
                eng.dma_start(out=xtt[g], in_=xt[:, lo : lo + xtt[g].shape[1]])

            t7 = _TILES - 1
            warm = res.tile([_P, 1], f32, name="warm")

            def sq(t):
                scrA = sp.tile([_P, a_cols], f8, tag="scrA", name=f"sa_{t}")
                if t < t7:
                    nc.scalar.activation(
                        scrA,
                        nat_ap(t, 0, a_cols),
                        AF.Square,
                        accum_out=st[:, t : t + 1],
                    )
                else:
                    nc.scalar.activation(
                        scrA[:, :a_mid],
                        nat_ap(t, 0, a_mid),
                        AF.Square,
                        accum_out=st[:, 7:8],
                    )
                    nc.scalar.activation(
                        scrA[:, a_mid:],
                        nat_ap(t, a_mid, a_cols),
                        AF.Square,
                        accum_out=st[:, 8:9],
                    )

            # scalar ring: few issues, ALL before ACT's compute so the
            # queue frees early; sync ring carries the rest in
            # consumption order.
            nc.scalar.dma_start(out=id_t, in_=ident[:, :])
            nat_dma(nc.scalar, 1, 0, 1, nat_cols)
            xt_dma(nc.scalar, 1)
            if variant.endswith("b"):
                nat_dma(nc.scalar, 3, 0, 3, nat_cols)
                xt_dma(nc.scalar, 3)
            nat_dma(nc.sync, 0, 0, 0, nat_cols)
            xt_dma(nc.sync, 0)
            nat_dma(nc.sync, 2, 0, 2, nat_cols)
            xt_dma(nc.sync, 2)
            if not variant.endswith("b"):
                nat_dma(nc.sync, 3, 0, 3, nat_cols)
                xt_dma(nc.sync, 3)
            nat_dma(nc.sync, 4, 0, 4, nat_cols)
            nat_dma(nc.sync, 5, 0, 5, nat_cols)
            nat_dma(nc.sync, 6, 0, 6, nat_cols)
            nat_dma(nc.sync, t7, 0, t7, a_mid)
            nat_dma(nc.sync, t7, a_mid, t7, d_mid)
            nat_dma(nc.sync, t7, d_mid, t7, nat_cols)
            nc.scalar.activation(warm, st[:, 0:1], AF.Square)
            for t in range(_TILES):
                sq(t)

            # ── TensorE ──
            for c in range(pe_chunks):
                g, l = c // GRP, c % GRP
                for rb in range(_TILES):
                    off = l * _RPC + rb * _P
                    sl = xtt[g][:, off : off + _P]
                    nc.tensor.matmul(
                        gram[rb][:, :_P],
                        sl,
                        sl,
                        start=(c == 0),
                        stop=(c == pe_chunks - 1),
                    )

            # ── VectorE ──
            def stt(t, lo, hi, sec):
                scrD = sp.tile([_P, hi - lo], f8, tag="scrD", name=f"sd_{sec}")
                nc.vector.scalar_tensor_tensor(
                    out=scrD,
                    in0=nat_ap(t, lo, hi),
                    scalar=1.0,
                    in1=nat_ap(t, lo, hi),
                    op0=ALU.mult,
                    op1=ALU.mult,
                    accum_out=st[:, sec : sec + 1],
                )

            for t in range(_TILES - 1):
                stt(t, a_cols, nat_cols, 9 + t)
            for rb in range(_TILES):
                dscr = sp.tile([_P, _P], bf16, tag="dscr", name=f"dg_{rb}")
                nc.vector.scalar_tensor_tensor(
                    out=dscr,
                    in0=gram[rb][:, :_P],
                    scalar=1.0,
                    in1=id_t,
                    op0=ALU.mult,
                    op1=ALU.mult,
                    accum_out=st[:, 18 + rb : 19 + rb],
                )
            nc.sync.dma_start(out=stats[:, 18:26], in_=st[:, 18:26])
            stt(t7, a_cols, d_mid, 16)
            stt(t7, d_mid, nat_cols, 17)
            nc.scalar.dma_start(out=stats[:, 0:9], in_=st[:, 0:9])
            nc.sync.dma_start(out=stats[:, 9:18], in_=st[:, 9:18])
    nc.finalize()
    return nc


def _get_nc(variant=None):
    if variant is None:
        variant = VARIANT
    if variant not in _NC_CACHE:
        if variant.startswith("hyb8"):
            _NC_CACHE[variant] = _build_hyb8(variant)
        elif variant.startswith(("hyb5", "hyb6", "hyb7")):
            _NC_CACHE[variant] = _build_hyb5(variant)
        elif variant.startswith("hyb4"):
            _NC_CACHE[variant] = _build_hyb4(variant)
        elif variant.startswith("hyb3"):
            _NC_CACHE[variant] = _build_hyb3(variant)
        elif variant.startswith("hyb2"):
            _NC_CACHE[variant] = _build_hyb2(variant)
        else:
            _NC_CACHE[variant] = _build_bass(variant)
    return _NC_CACHE[variant]


def _make_in_maps(x, variant=None):
    """x: [BATCH, N] float32 -> per-core input dicts (fp8 cast here)."""
    import ml_dtypes

    if variant is None:
        variant = VARIANT
    a_cols, d_cols, pe_chunks = _params(variant)
    nat_cols = a_cols + d_cols
    x8 = x.astype(ml_dtypes.float8_e4m3)
    id128 = np.eye(_P, dtype=ml_dtypes.bfloat16)
    maps = []
    for i in range(_NCORES):
        xs = x8[i * _RPC : (i + 1) * _RPC]
        if variant.startswith(("hyb5", "hyb6", "hyb7", "hyb8")):
            # xn2[p, t*nat+c] = xs[t*128+p, c]
            nat = xs[:, :nat_cols].reshape(_TILES, _P, nat_cols)
            m = {
                "xn2": np.ascontiguousarray(
                    nat.transpose(1, 0, 2).reshape(_P, _TILES * nat_cols)
                )
            }
        else:
            m = {"xn": np.ascontiguousarray(xs[:, :nat_cols])}
        if pe_chunks:
            # packed[p, c*RPC + r] = xs[r, nat+c*128+p]
            pe = xs[:, nat_cols:].reshape(_RPC, pe_chunks, _P)
            m["xt"] = np.ascontiguousarray(
                pe.transpose(2, 1, 0).reshape(_P, pe_chunks * _RPC)
            )
            m["ident"] = id128
        maps.append(m)
    return maps


def _exact_p_y(xrows, yrows):
    """f64 exact solve of the knapsack dual for fallback rows."""
    xr = np.asarray(xrows, dtype=np.float64)
    if xr.ndim == 1:
        xr = xr[None, :]
    n = xr.shape[1]
    norm = np.maximum(np.sqrt((xr * xr).sum(1, keepdims=True)), 1e-12)
    e = xr / norm / _TAU
    lo = e.min(1) - _EPS
    hi = e.max(1) + _EPS * np.log(float(n))
    for _ in range(200):
        mid = 0.5 * (lo + hi)
        f = np.minimum(1.0, np.exp((e - mid[:, None]) / _EPS - 1.0)).sum(1)
        big = f > _K
        lo = np.where(big, mid, lo)
        hi = np.where(big, hi, mid)
    nu = 0.5 * (lo + hi)
    e_y = e[np.arange(e.shape[0]), yrows]
    return np.minimum(1.0, np.exp((e_y - nu) / _EPS - 1.0))


def kernel(x, y):
    from concourse.bass_utils import run_bass_kernel_spmd

    x = np.asarray(x, dtype=np.float32)
    y = np.asarray(y).astype(np.int64)
    assert x.shape == (_BATCH, _N)

    nc = _get_nc()
    in_maps = _make_in_maps(x)
    res = run_bass_kernel_spmd(nc, in_maps, core_ids=list(range(_NCORES)))

    parts = []
    for r in res.results:
        st = r["stats"].astype(np.float64)
        if VARIANT.startswith(("hyb2", "hyb3", "hyb4", "hyb5", "hyb6", "hyb7", "hyb8")):
            # cols 0-7 ACT (7=t7a), 8 ACT t7b, 9-16 DVE (16=t7a),
            # 17 DVE t7b, 18-25 PE blocks
            s2t = st[:, 0:8] + st[:, 9:17] + st[:, 18:26]  # [P, TILES]
            s2t[:, 7] += st[:, 8] + st[:, 17]
            s2 = s2t.T.reshape(-1)  # row t*128+p
        else:
            k = st.shape[1] // _TILES
            # S2 for row (t*128+p) = sum_sec st[p, sec*TILES + t]
            s2 = st.reshape(_P, k, _TILES).sum(axis=1).T.reshape(-1)
        parts.append(s2)
    S2 = np.concatenate(parts)  # [BATCH]

    rows = np.arange(_BATCH)
    x_y = x[rows, y].astype(np.float64)
    with np.errstate(all="ignore"):
        c = 1.0 / (np.sqrt(S2) * _TAU)
        e_y = x_y * c
        s = float(_N) + 0.5  # N + c*S1(dropped) + 0.5*c^2*S2 (== 0.5)
        p_y = np.minimum(1.0, _K * np.exp(e_y) / s)
        bad = ~(np.isfinite(p_y) & (S2 > 0))
    if bad.any():
        p_y[bad] = _exact_p_y(x[bad], y[bad])
    loss = np.mean(-np.log(p_y + 1e-8))
    return np.array(loss, dtype=np.float32)


# revision 35
# speedup vs baseline: 1.0393x; 1.0393x over previous
"""Trainium2 Bass kernel for the entropy-regularized knapsack CVX loss.

Math: with e = x / (||x||_2 * TAU), the per-row solution of
    max e@z + EPS*sum(entr(z))  s.t. 0<=z<=1, sum z = K
is p_i = min(1, exp((e_i - nu)/EPS - 1)) with nu s.t. sum_i p_i = K.
Since |e_i| <= 1 (Cauchy-Schwarz) and n = 8192 >> K*e^2, the min(1,.)
clamp is never active at the optimum, so p = K * softmax(e) and
loss = mean(-log(K*exp(e_y)/s + 1e-8)) with s = sum_j exp(e_j).

Key reduction: ||e||_2 = 1/TAU = 1, so the 2nd-order Taylor expansion of
s around 0 is UNCONDITIONALLY accurate:
    s = sum exp(e_j) = N + sum e_j + 0.5*sum e_j^2 + R,
    |R| <= e/6 * (sum e_j^2)^{3/2} ~ 0.45 abs  (vs s ~ N = 8192),
i.e. rel err <= 5.6e-5 for ANY row; sum e_j^2 = 1 exactly.  The linear
term (~1e-4 relative for real data, <= 1.1% worst-case) is dropped --
validated: loss rel err vs reference ~1e-7 (tolerance 2e-2).

So the DEVICE only needs the per-row sum of squares S2 = sum_j x_ij^2
(norm and quadratic term in one).  Host does the O(B) rest: gather
x[r, y[r]], p_y = K*exp(x_y/sqrt(S2))/(N + 0.5), loss mean.

Device kernel (data-parallel over 8 cores, 1024 rows each, fp8 input):
three engines square-reduce disjoint column ranges in parallel so the
kernel rides the 8 MB/core fp8 DMA roofline (~23.5 us @ 358 GB/s):
  - ScalarE:  cols [0, A)        Square activation + fused accum
  - VectorE:  cols [A, A+D)      scalar_tensor_tensor (x*1)*x + accum
  - TensorE:  cols [A+D, 8192)   host-transposed 128-col chunks; for
    each 128-row block rb, matmul(lhsT=xT_chunk[:, rb], rhs=same)
    accumulates the Gram block of rows rb into PSUM bank rb; the
    diagonal (= sum of squares) is pulled out by one identity-masked
    scalar_tensor_tensor with accum_out per bank.  Effective rate
    ~0.63 ns per column-of-all-rows -- faster than ACT's 0.90.
DMAs are interleaved (xt group / nat tile) in consumption order with
bufs-bounded pools so all three engines stream without startup stalls.
fp8 quantization only perturbs the NORM (the host computes e_y from
full-precision x): S2 rel err ~0.1% -> loss rel err ~1e-7 (validated).
Exact f64 fallback for any row with nonfinite/nonpositive S2.
"""

import numpy as np

_BATCH = 8192
_N = 8192
_NCORES = 8
_RPC = _BATCH // _NCORES  # rows per core
_P = 128
_TILES = _RPC // _P  # row-tiles (and PE row-blocks) per core
_K = 5.0
_TAU = 1.0
_EPS = 1.0

_NC_CACHE = {}
VARIANT = "hyb8"

# (act_cols, dve_cols, pe_chunks): column split per 8192-wide row set.
# HW rates: ACT (A+352)/1.2 ns + 186/tile, DVE-STT (D+151)/0.96 ns,
# PE ~56 ns warm per (128-col chunk, 128-row block) LDW+MM pair.
_SPLITS = {
    "hyb": (2432, 2176, 28),
    "hyb2": (2816, 2048, 26),
    "hyb3": (2304, 1920, 31),
    "hyb4": (2304, 1920, 31),
    "hyb4g": (2304, 1920, 31),  # xt stream via gpsimd SWDGE ring
    "hyb5": (2304, 1920, 31),
    "hyb6": (2304, 1920, 31),
    "hyb6b": (1920, 2176, 32),
    "hyb6g": (2304, 1920, 31),  # probe: all DMAs on the sync ring
    "hyb7": (2304, 1920, 31),
    "hyb8": (2176, 1920, 32),
    "hyb8b": (2176, 1920, 32),
    "sq8": (4480, 3712, 0),  # fallback: no PE (old baseline split)
}


def _params(variant):
    a_cols, d_cols, pe_chunks = _SPLITS[variant]
    assert a_cols + d_cols + pe_chunks * _P == _N
    return a_cols, d_cols, pe_chunks


def _build_bass(variant=None):
    import concourse.bacc as bacc
    import concourse.mybir as mybir
    import concourse.tile as tile

    if variant is None:
        variant = VARIANT
    a_cols, d_cols, pe_chunks = _params(variant)
    nat_cols = a_cols + d_cols
    # xt groups of 4 chunks (one DMA each)
    GRP = 4
    n_grp = (pe_chunks + GRP - 1) // GRP

    nc = bacc.Bacc(
        "TRN2", target_bir_lowering=False, debug=False, num_devices=_NCORES
    )
    f32 = mybir.dt.float32
    bf16 = mybir.dt.bfloat16
    f8 = mybir.dt.float8e4
    AF = mybir.ActivationFunctionType
    ALU = mybir.AluOpType

    xn = nc.dram_tensor("xn", [_RPC, nat_cols], f8, kind="ExternalInput")
    if pe_chunks:
        # packed transposed chunks: xt[p, c*RPC + r] = x[r, nat+c*128+p]
        xt = nc.dram_tensor(
            "xt", [_P, pe_chunks * _RPC], f8, kind="ExternalInput"
        )
        ident = nc.dram_tensor("ident", [_P, _P], bf16, kind="ExternalInput")
    k_st = 3 if pe_chunks else 2
    stats = nc.dram_tensor("stats", [_P, k_st * _TILES], f32, kind="ExternalOutput")

    with tile.TileContext(nc) as tc:
        with (
            tc.tile_pool(name="xnp", bufs=3) as xnp,
            tc.tile_pool(name="xtp", bufs=3) as xtp,
            tc.tile_pool(name="sp", bufs=2) as sp,
            tc.tile_pool(name="singles", bufs=1) as singles,
            tc.tile_pool(name="psum", bufs=1, space="PSUM") as psp,
        ):
            stA = singles.tile([_P, _TILES], f32, name="stA")
            stD = singles.tile([_P, _TILES], f32, name="stD")
            nc.vector.memset(stA, 0.0)
            nc.vector.memset(stD, 0.0)
            if pe_chunks:
                stG = singles.tile([_P, _TILES], f32, name="stG")
                nc.vector.memset(stG, 0.0)
                id_t = singles.tile([_P, _P], bf16, name="id_t")
                nc.sync.dma_start(out=id_t, in_=ident[:, :])
                gram = [
                    psp.tile([_P, 512], f32, name=f"gram_{rb}")
                    for rb in range(_TILES)
                ]
                # Dummy 1-elem Square hoists the ACT table load so it
                # overlaps the head DMAs instead of the first real op.
                warm = singles.tile([_P, 1], f32, name="warm")
                nc.scalar.activation(warm, id_t[:, 0:1], AF.Square)
            else:
                warm = singles.tile([_P, 1], f32, name="warm")
                nc.scalar.activation(warm, stA[:, 0:1], AF.Square)

            def nat_tile(t):
                x_tile = xnp.tile([_P, nat_cols], f8, tag="xn", name=f"xn_{t}")
                nc.sync.dma_start(out=x_tile, in_=xn[t * _P : (t + 1) * _P, :])
                scrA = sp.tile([_P, a_cols], f8, tag="scrA", name=f"sa_{t}")
                nc.scalar.activation(
                    scrA,
                    x_tile[:, :a_cols],
                    AF.Square,
                    accum_out=stA[:, t : t + 1],
                )
                scrD = sp.tile([_P, d_cols], f8, tag="scrD", name=f"sd_{t}")
                nc.vector.scalar_tensor_tensor(
                    out=scrD,
                    in0=x_tile[:, a_cols:],
                    scalar=1.0,
                    in1=x_tile[:, a_cols:],
                    op0=ALU.mult,
                    op1=ALU.mult,
                    accum_out=stD[:, t : t + 1],
                )

            def xt_group(g):
                lo = g * GRP
                hi = min(lo + GRP, pe_chunks)
                w = (hi - lo) * _RPC
                xt_t = xtp.tile([_P, w], f8, tag="xt", name=f"xt_{g}")
                nc.sync.dma_start(
                    out=xt_t, in_=xt[:, lo * _RPC : lo * _RPC + w]
                )
                for l in range(hi - lo):
                    c = lo + l
                    for rb in range(_TILES):
                        off = l * _RPC + rb * _P
                        sl = xt_t[:, off : off + _P]
                        nc.tensor.matmul(
                            gram[rb][:, :_P],
                            sl,
                            sl,
                            start=(c == 0),
                            stop=(c == pe_chunks - 1),
                        )

            # Interleave DMA/compute issue in consumption order: PE first
            # (it can start before ACT's table load finishes), then
            # alternate nat tiles and xt groups.
            if pe_chunks:
                xt_group(0)
            nat_tile(0)
            for i in range(1, max(n_grp, _TILES)):
                if pe_chunks and i < n_grp:
                    xt_group(i)
                if i < _TILES:
                    nat_tile(i)

            if pe_chunks:
                # diag(gram[rb]) via identity-masked STT, fused accum.
                for rb in range(_TILES):
                    dscr = sp.tile([_P, _P], bf16, tag="dscr", name=f"dg_{rb}")
                    nc.vector.scalar_tensor_tensor(
                        out=dscr,
                        in0=gram[rb][:, :_P],
                        scalar=1.0,
                        in1=id_t,
                        op0=ALU.mult,
                        op1=ALU.mult,
                        accum_out=stG[:, rb : rb + 1],
                    )

            nc.sync.dma_start(out=stats[:, 0:_TILES], in_=stA)
            nc.sync.dma_start(out=stats[:, _TILES : 2 * _TILES], in_=stD)
            if pe_chunks:
                nc.sync.dma_start(out=stats[:, 2 * _TILES :], in_=stG)
    nc.finalize()
    return nc


def _build_hyb2(variant="hyb2"):
    """v2: express-lane head DMAs on the ACT HWDGE ring, deep-buffered
    bulk stream on the SP ring, last tile split into sub-chunks with
    separate accumulator sections, per-engine contiguous stats DMAs."""
    import concourse.bacc as bacc
    import concourse.mybir as mybir
    import concourse.tile as tile

    a_cols, d_cols, pe_chunks = _params(variant)
    nat_cols = a_cols + d_cols
    n_grp = (pe_chunks + 3) // 4  # 4-chunk groups; last may be short

    nc = bacc.Bacc(
        "TRN2", target_bir_lowering=False, debug=False, num_devices=_NCORES
    )
    f32 = mybir.dt.float32
    bf16 = mybir.dt.bfloat16
    f8 = mybir.dt.float8e4
    AF = mybir.ActivationFunctionType
    ALU = mybir.AluOpType

    xn = nc.dram_tensor("xn", [_RPC, nat_cols], f8, kind="ExternalInput")
    xt = nc.dram_tensor("xt", [_P, pe_chunks * _RPC], f8, kind="ExternalInput")
    ident = nc.dram_tensor("ident", [_P, _P], bf16, kind="ExternalInput")
    # cols 0-7 ACT tiles (7 = sub-op a), 8 ACT t7 sub-op b,
    # 9-16 DVE tiles (16 = sub-op a), 17 DVE t7 sub-op b, 18-25 PE blocks
    NSEC = 26
    stats = nc.dram_tensor("stats", [_P, NSEC], f32, kind="ExternalOutput")

    # t7 sub-chunk boundaries (within nat cols)
    a_mid = 2048
    d_mid = a_cols + ((d_cols * 3) // 4 // _P) * _P  # last DVE chunk small

    with tile.TileContext(nc) as tc:
        with (
            tc.tile_pool(name="xnp", bufs=4) as xnp,
            tc.tile_pool(name="xtp", bufs=3) as xtp,
            tc.tile_pool(name="sp", bufs=2) as sp,
            tc.tile_pool(name="singles", bufs=1) as singles,
            tc.tile_pool(name="psum", bufs=1, space="PSUM") as psp,
        ):
            st = singles.tile([_P, NSEC], f32, name="st")
            nc.vector.memset(st, 0.0)
            id_t = singles.tile([_P, _P], bf16, name="id_t")
            gram = [
                psp.tile([_P, 512], f32, name=f"gram_{rb}")
                for rb in range(_TILES)
            ]

            nat = [
                xnp.tile([_P, nat_cols], f8, tag="xn", name=f"xn_{t}")
                for t in range(_TILES)
            ]
            xtt = []
            for g in range(n_grp):
                w = (min((g + 1) * 4, pe_chunks) - g * 4) * _RPC
                xtt.append(xtp.tile([_P, w], f8, tag="xt", name=f"xt_{g}"))

            # ── express head (ACT HWDGE ring): ident, xt chunks 0-3,
            # nat tiles 0-1 split per engine section ──
            nc.scalar.dma_start(out=id_t, in_=ident[:, :])
            nc.scalar.dma_start(
                out=xtt[0][:, :_RPC], in_=xt[:, 0:_RPC]
            )
            for t in (0, 1):
                nc.scalar.dma_start(
                    out=nat[t][:, :a_cols],
                    in_=xn[t * _P : (t + 1) * _P, :a_cols],
                )
                nc.scalar.dma_start(
                    out=nat[t][:, a_cols:],
                    in_=xn[t * _P : (t + 1) * _P, a_cols:],
                )
                if t == 0:
                    for c in (1, 2, 3):
                        nc.scalar.dma_start(
                            out=xtt[0][:, c * _RPC : (c + 1) * _RPC],
                            in_=xt[:, c * _RPC : (c + 1) * _RPC],
                        )

            # ── bulk stream (SP ring), xt-leading interleave ──
            for i in range(1, max(n_grp, _TILES - 1)):
                if i < n_grp:
                    lo = i * 4 * _RPC
                    nc.sync.dma_start(
                        out=xtt[i], in_=xt[:, lo : lo + xtt[i].shape[1]]
                    )
                t = i + 1
                if 2 <= t < _TILES - 1:
                    nc.sync.dma_start(
                        out=nat[t], in_=xn[t * _P : (t + 1) * _P, :]
                    )
            # last tile in 4 sub-chunks (small final arrivals)
            t = _TILES - 1
            r0 = t * _P
            for lo, hi in (
                (0, a_mid),
                (a_mid, a_cols),
                (a_cols, d_mid),
                (d_mid, nat_cols),
            ):
                nc.sync.dma_start(
                    out=nat[t][:, lo:hi], in_=xn[r0 : r0 + _P, lo:hi]
                )

            # ── ScalarE: table-load warm + squares ──
            warm = singles.tile([_P, 1], f32, name="warm")
            nc.scalar.activation(warm, id_t[:, 0:1], AF.Square)
            for t in range(_TILES):
                if t < _TILES - 1:
                    scrA = sp.tile([_P, a_cols], f8, tag="scrA", name=f"sa_{t}")
                    nc.scalar.activation(
                        scrA,
                        nat[t][:, :a_cols],
                        AF.Square,
                        accum_out=st[:, t : t + 1],
                    )
                else:
                    scrA = sp.tile([_P, a_cols], f8, tag="scrA", name=f"sa_{t}")
                    nc.scalar.activation(
                        scrA[:, :a_mid],
                        nat[t][:, :a_mid],
                        AF.Square,
                        accum_out=st[:, 7:8],
                    )
                    nc.scalar.activation(
                        scrA[:, a_mid:],
                        nat[t][:, a_mid:a_cols],
                        AF.Square,
                        accum_out=st[:, 8:9],
                    )
            nc.scalar.dma_start(out=stats[:, 0:9], in_=st[:, 0:9])

            # ── TensorE: Gram-block accumulation, chunk-major ──
            for c in range(pe_chunks):
                g, l = c // 4, c % 4
                for rb in range(_TILES):
                    off = l * _RPC + rb * _P
                    sl = xtt[g][:, off : off + _P]
                    nc.tensor.matmul(
                        gram[rb][:, :_P],
                        sl,
                        sl,
                        start=(c == 0),
                        stop=(c == pe_chunks - 1),
                    )

            # ── VectorE: STT squares; diags fill the last-tile DMA wait ──
            def stt(t, lo, hi, sec):
                scrD = sp.tile(
                    [_P, hi - lo], f8, tag="scrD", name=f"sd_{sec}"
                )
                nc.vector.scalar_tensor_tensor(
                    out=scrD,
                    in0=nat[t][:, lo:hi],
                    scalar=1.0,
                    in1=nat[t][:, lo:hi],
                    op0=ALU.mult,
                    op1=ALU.mult,
                    accum_out=st[:, sec : sec + 1],
                )

            for t in range(_TILES - 1):
                stt(t, a_cols, nat_cols, 9 + t)
            for rb in range(_TILES):
                dscr = sp.tile([_P, _P], bf16, tag="dscr", name=f"dg_{rb}")
                nc.vector.scalar_tensor_tensor(
                    out=dscr,
                    in0=gram[rb][:, :_P],
                    scalar=1.0,
                    in1=id_t,
                    op0=ALU.mult,
                    op1=ALU.mult,
                    accum_out=st[:, 18 + rb : 19 + rb],
                )
            nc.sync.dma_start(out=stats[:, 18:26], in_=st[:, 18:26])
            stt(_TILES - 1, a_cols, d_mid, 16)
            stt(_TILES - 1, d_mid, nat_cols, 17)
            nc.sync.dma_start(out=stats[:, 9:18], in_=st[:, 9:18])
    nc.finalize()
    return nc


def _build_hyb3(variant="hyb3"):
    """v3: everything resident in SBUF (no pool recycling) so every DMA
    is issued up front and the stream runs at the HBM roofline start to
    finish; first-needed chunks issued first in small pieces so compute
    starts early; last tile in small sub-chunks to shrink the tail."""
    import concourse.bacc as bacc
    import concourse.mybir as mybir
    import concourse.tile as tile

    a_cols, d_cols, pe_chunks = _params(variant)
    nat_cols = a_cols + d_cols
    n_grp = (pe_chunks + 3) // 4

    nc = bacc.Bacc(
        "TRN2", target_bir_lowering=False, debug=False, num_devices=_NCORES
    )
    f32 = mybir.dt.float32
    bf16 = mybir.dt.bfloat16
    f8 = mybir.dt.float8e4
    AF = mybir.ActivationFunctionType
    ALU = mybir.AluOpType

    xn = nc.dram_tensor("xn", [_RPC, nat_cols], f8, kind="ExternalInput")
    xt = nc.dram_tensor("xt", [_P, pe_chunks * _RPC], f8, kind="ExternalInput")
    ident = nc.dram_tensor("ident", [_P, _P], bf16, kind="ExternalInput")
    NSEC = 26
    stats = nc.dram_tensor("stats", [_P, NSEC], f32, kind="ExternalOutput")

    a_mid = (a_cols * 3 // 4 // _P) * _P  # t7 ACT sub-split (small 2nd op)
    d_mid = a_cols + (d_cols * 3 // 4 // _P) * _P

    with tile.TileContext(nc) as tc:
        with (
            tc.tile_pool(name="res", bufs=1) as res,
            tc.tile_pool(name="sp", bufs=2) as sp,
            tc.tile_pool(name="psum", bufs=1, space="PSUM") as psp,
        ):
            st = res.tile([_P, NSEC], f32, name="st")
            nc.vector.memset(st, 0.0)
            id_t = res.tile([_P, _P], bf16, name="id_t")
            gram = [
                psp.tile([_P, 512], f32, name=f"gram_{rb}")
                for rb in range(_TILES)
            ]
            nat = [
                res.tile([_P, nat_cols], f8, name=f"xn_{t}")
                for t in range(_TILES)
            ]
            xtt = [
                res.tile(
                    [_P, (min((g + 1) * 4, pe_chunks) - g * 4) * _RPC],
                    f8,
                    name=f"xt_{g}",
                )
                for g in range(n_grp)
            ]

            def nat_dma(t, lo, hi):
                nc.sync.dma_start(
                    out=nat[t][:, lo:hi],
                    in_=xn[t * _P : (t + 1) * _P, lo:hi],
                )

            # head: tile-0 engine sections in halves, then xt chunk 0-3,
            # then ident (needed ~7us for table-load warm)
            nat_dma(0, 0, a_cols // 2)
            nat_dma(0, a_cols // 2, a_cols)
            nat_dma(0, a_cols, a_cols + d_cols // 2)
            nat_dma(0, a_cols + d_cols // 2, nat_cols)
            for c in range(4):
                nc.sync.dma_start(
                    out=xtt[0][:, c * _RPC : (c + 1) * _RPC],
                    in_=xt[:, c * _RPC : (c + 1) * _RPC],
                )
            nc.sync.dma_start(out=id_t, in_=ident[:, :])
            nat_dma(1, 0, a_cols)
            nat_dma(1, a_cols, nat_cols)
            # bulk: xt-leading alternation, whole transfers
            for i in range(1, max(n_grp, _TILES - 1)):
                if i < n_grp:
                    lo = i * 4 * _RPC
                    nc.sync.dma_start(
                        out=xtt[i], in_=xt[:, lo : lo + xtt[i].shape[1]]
                    )
                t = i + 1
                if 2 <= t < _TILES - 1:
                    nat_dma(t, 0, nat_cols)
            # tail: last tile in 4 sub-chunks, smallest last
            t = _TILES - 1
            nat_dma(t, 0, a_mid)
            nat_dma(t, a_mid, a_cols)
            nat_dma(t, a_cols, d_mid)
            nat_dma(t, d_mid, nat_cols)

            # ── ScalarE ──
            warm = res.tile([_P, 1], f32, name="warm")
            nc.scalar.activation(warm, id_t[:, 0:1], AF.Square)
            for t in range(_TILES):
                scrA = sp.tile([_P, a_cols], f8, tag="scrA", name=f"sa_{t}")
                if t < _TILES - 1:
                    nc.scalar.activation(
                        scrA,
                        nat[t][:, :a_cols],
                        AF.Square,
                        accum_out=st[:, t : t + 1],
                    )
                else:
                    nc.scalar.activation(
                        scrA[:, :a_mid],
                        nat[t][:, :a_mid],
                        AF.Square,
                        accum_out=st[:, 7:8],
                    )
                    nc.scalar.activation(
                        scrA[:, a_mid:],
                        nat[t][:, a_mid:a_cols],
                        AF.Square,
                        accum_out=st[:, 8:9],
                    )

            # ── TensorE ──
            for c in range(pe_chunks):
                g, l = c // 4, c % 4
                for rb in range(_TILES):
                    off = l * _RPC + rb * _P
                    sl = xtt[g][:, off : off + _P]
                    nc.tensor.matmul(
                        gram[rb][:, :_P],
                        sl,
                        sl,
                        start=(c == 0),
                        stop=(c == pe_chunks - 1),
                    )

            # ── VectorE ──
            def stt(t, lo, hi, sec):
                scrD = sp.tile([_P, hi - lo], f8, tag="scrD", name=f"sd_{sec}")
                nc.vector.scalar_tensor_tensor(
                    out=scrD,
                    in0=nat[t][:, lo:hi],
                    scalar=1.0,
                    in1=nat[t][:, lo:hi],
                    op0=ALU.mult,
                    op1=ALU.mult,
                    accum_out=st[:, sec : sec + 1],
                )

            for t in range(_TILES - 1):
                stt(t, a_cols, nat_cols, 9 + t)
            for rb in range(_TILES):
                dscr = sp.tile([_P, _P], bf16, tag="dscr", name=f"dg_{rb}")
                nc.vector.scalar_tensor_tensor(
                    out=dscr,
                    in0=gram[rb][:, :_P],
                    scalar=1.0,
                    in1=id_t,
                    op0=ALU.mult,
                    op1=ALU.mult,
                    accum_out=st[:, 18 + rb : 19 + rb],
                )
            nc.sync.dma_start(out=stats[:, 18:26], in_=st[:, 18:26])
            nc.scalar.dma_start(out=stats[:, 0:9], in_=st[:, 0:9])
            stt(_TILES - 1, a_cols, d_mid, 16)
            stt(_TILES - 1, d_mid, nat_cols, 17)
            nc.sync.dma_start(out=stats[:, 9:18], in_=st[:, 9:18])
    nc.finalize()
    return nc


def _build_hyb4(variant="hyb4"):
    """v4: all tiles resident (DMA fully decoupled from compute), large
    ~1MB transfers for descriptor efficiency, diag extraction placed to
    overlap the last nat tile's DMA wait, small final sub-chunks."""
    import concourse.bacc as bacc
    import concourse.mybir as mybir
    import concourse.tile as tile

    a_cols, d_cols, pe_chunks = _params(variant)
    nat_cols = a_cols + d_cols
    xt_dma = nc_dma = None  # set below

    nc = bacc.Bacc(
        "TRN2", target_bir_lowering=False, debug=False, num_devices=_NCORES
    )
    f32 = mybir.dt.float32
    bf16 = mybir.dt.bfloat16
    f8 = mybir.dt.float8e4
    AF = mybir.ActivationFunctionType
    ALU = mybir.AluOpType

    xt_engine = nc.gpsimd if variant.endswith("g") else nc.sync

    xn = nc.dram_tensor("xn", [_RPC, nat_cols], f8, kind="ExternalInput")
    xt = nc.dram_tensor("xt", [_P, pe_chunks * _RPC], f8, kind="ExternalInput")
    ident = nc.dram_tensor("ident", [_P, _P], bf16, kind="ExternalInput")
    NSEC = 26
    stats = nc.dram_tensor("stats", [_P, NSEC], f32, kind="ExternalOutput")

    a_mid = (a_cols * 3 // 4 // _P) * _P
    d_mid = a_cols + (d_cols * 3 // 4 // _P) * _P

    # xt transfer groups: 8 chunks (1 MB) each, last group short
    GRP = 8
    n_grp = (pe_chunks + GRP - 1) // GRP

    with tile.TileContext(nc) as tc:
        with (
            tc.tile_pool(name="res", bufs=1) as res,
            tc.tile_pool(name="sp", bufs=2) as sp,
            tc.tile_pool(name="psum", bufs=1, space="PSUM") as psp,
        ):
            st = res.tile([_P, NSEC], f32, name="st")
            nc.vector.memset(st, 0.0)
            id_t = res.tile([_P, _P], bf16, name="id_t")
            gram = [
                psp.tile([_P, 512], f32, name=f"gram_{rb}")
                for rb in range(_TILES)
            ]
            nat = [
                res.tile([_P, nat_cols], f8, name=f"xn_{t}")
                for t in range(_TILES)
            ]
            xtt = [
                res.tile(
                    [_P, (min((g + 1) * GRP, pe_chunks) - g * GRP) * _RPC],
                    f8,
                    name=f"xt_{g}",
                )
                for g in range(n_grp)
            ]

            def nat_dma(t, lo, hi):
                nc.sync.dma_start(
                    out=nat[t][:, lo:hi],
                    in_=xn[t * _P : (t + 1) * _P, lo:hi],
                )

            def xt_dma(g):
                lo = g * GRP * _RPC
                xt_engine.dma_start(
                    out=xtt[g], in_=xt[:, lo : lo + xtt[g].shape[1]]
                )

            # DMA issue order = rough consumption order; queue-FIFO
            # stacking keeps the stream saturated to the end.
            nc.sync.dma_start(out=id_t, in_=ident[:, :])
            nat_dma(0, 0, nat_cols)
            xt_dma(0)
            nat_dma(1, 0, nat_cols)
            xt_dma(1)
            nat_dma(2, 0, nat_cols)
            xt_dma(2)
            nat_dma(3, 0, nat_cols)
            if n_grp > 3:
                xt_dma(3)
            for t in range(4, _TILES - 1):
                nat_dma(t, 0, nat_cols)
            t = _TILES - 1
            nat_dma(t, 0, a_mid)
            nat_dma(t, a_mid, a_cols)
            nat_dma(t, a_cols, d_mid)
            nat_dma(t, d_mid, nat_cols)

            # ── ScalarE ──
            warm = res.tile([_P, 1], f32, name="warm")
            nc.scalar.activation(warm, st[:, 0:1], AF.Square)
            for t in range(_TILES):
                scrA = sp.tile([_P, a_cols], f8, tag="scrA", name=f"sa_{t}")
                if t < _TILES - 1:
                    nc.scalar.activation(
                        scrA,
                        nat[t][:, :a_cols],
                        AF.Square,
                        accum_out=st[:, t : t + 1],
                    )
                else:
                    nc.scalar.activation(
                        scrA[:, :a_mid],
                        nat[t][:, :a_mid],
                        AF.Square,
                        accum_out=st[:, 7:8],
                    )
                    nc.scalar.activation(
                        scrA[:, a_mid:],
                        nat[t][:, a_mid:a_cols],
                        AF.Square,
                        accum_out=st[:, 8:9],
                    )

            # ── TensorE ──
            for c in range(pe_chunks):
                g, l = c // GRP, c % GRP
                for rb in range(_TILES):
                    off = l * _RPC + rb * _P
                    sl = xtt[g][:, off : off + _P]
                    nc.tensor.matmul(
                        gram[rb][:, :_P],
                        sl,
                        sl,
                        start=(c == 0),
                        stop=(c == pe_chunks - 1),
                    )

            # ── VectorE ──
            def stt(t, lo, hi, sec):
                scrD = sp.tile([_P, hi - lo], f8, tag="scrD", name=f"sd_{sec}")
                nc.vector.scalar_tensor_tensor(
                    out=scrD,
                    in0=nat[t][:, lo:hi],
                    scalar=1.0,
                    in1=nat[t][:, lo:hi],
                    op0=ALU.mult,
                    op1=ALU.mult,
                    accum_out=st[:, sec : sec + 1],
                )

            for t in range(_TILES - 1):
                stt(t, a_cols, nat_cols, 9 + t)
            for rb in range(_TILES):
                dscr = sp.tile([_P, _P], bf16, tag="dscr", name=f"dg_{rb}")
                nc.vector.scalar_tensor_tensor(
                    out=dscr,
                    in0=gram[rb][:, :_P],
                    scalar=1.0,
                    in1=id_t,
                    op0=ALU.mult,
                    op1=ALU.mult,
                    accum_out=st[:, 18 + rb : 19 + rb],
                )
            nc.sync.dma_start(out=stats[:, 18:26], in_=st[:, 18:26])
            stt(_TILES - 1, a_cols, d_mid, 16)
            stt(_TILES - 1, d_mid, nat_cols, 17)
            nc.scalar.dma_start(out=stats[:, 0:9], in_=st[:, 0:9])
            nc.sync.dma_start(out=stats[:, 9:18], in_=st[:, 9:18])
    nc.finalize()
    return nc


def _build_hyb5(variant="hyb5"):
    """v5: dma_start issue costs ~650ns serialized per sequencer and the
    HWDGE ring caps ~10 in-flight transfers, so use FEW large DMAs and
    split them across two independent issue paths: sync HWDGE carries
    the host-packed natural stream, gpsimd SWDGE carries the transposed
    PE stream.  All tiles resident; compute slices one big nat tile."""
    import concourse.bacc as bacc
    import concourse.mybir as mybir
    import concourse.tile as tile

    a_cols, d_cols, pe_chunks = _params(variant)
    nat_cols = a_cols + d_cols

    nc = bacc.Bacc(
        "TRN2", target_bir_lowering=False, debug=False, num_devices=_NCORES
    )
    f32 = mybir.dt.float32
    bf16 = mybir.dt.bfloat16
    f8 = mybir.dt.float8e4
    AF = mybir.ActivationFunctionType
    ALU = mybir.AluOpType

    # xn2[p, t*nat+c] = x[t*128+p, c]  (host-packed row-tile-major)
    xn = nc.dram_tensor(
        "xn2", [_P, _TILES * nat_cols], f8, kind="ExternalInput"
    )
    xt = nc.dram_tensor("xt", [_P, pe_chunks * _RPC], f8, kind="ExternalInput")
    ident = nc.dram_tensor("ident", [_P, _P], bf16, kind="ExternalInput")
    NSEC = 26
    stats = nc.dram_tensor("stats", [_P, NSEC], f32, kind="ExternalOutput")

    a_mid = (a_cols * 3 // 4 // _P) * _P
    d_mid = a_cols + (d_cols * 3 // 4 // _P) * _P

    GRP = 8
    n_grp = (pe_chunks + GRP - 1) // GRP

    with tile.TileContext(nc) as tc:
        with (
            tc.tile_pool(name="res", bufs=1) as res,
            tc.tile_pool(name="sp", bufs=2) as sp,
            tc.tile_pool(name="psum", bufs=1, space="PSUM") as psp,
        ):
            st = res.tile([_P, NSEC], f32, name="st")
            nc.vector.memset(st, 0.0)
            id_t = res.tile([_P, _P], bf16, name="id_t")
            gram = [
                psp.tile([_P, 512], f32, name=f"gram_{rb}")
                for rb in range(_TILES)
            ]
            xna = res.tile([_P, _TILES * nat_cols], f8, name="xna")
            xtt = [
                res.tile(
                    [_P, (min((g + 1) * GRP, pe_chunks) - g * GRP) * _RPC],
                    f8,
                    name=f"xt_{g}",
                )
                for g in range(n_grp)
            ]

            def nat_ap(t, lo, hi):
                return xna[:, t * nat_cols + lo : t * nat_cols + hi]

            def nat_dma(eng, lo_t, lo, hi_t, hi):
                a, b = lo_t * nat_cols + lo, hi_t * nat_cols + hi
                eng.dma_start(out=xna[:, a:b], in_=xn[:, a:b])

            def xt_dma(eng, g):
                lo = g * GRP * _RPC
                eng.dma_start(out=xtt[g], in_=xt[:, lo : lo + xtt[g].shape[1]])

            t7 = _TILES - 1
            if variant.endswith("g"):
                # probe: everything on the sync ring
                nc.sync.dma_start(out=id_t, in_=ident[:, :])
                for t in range(_TILES - 1):
                    nat_dma(nc.sync, t, 0, t, nat_cols)
                for g in range(n_grp):
                    xt_dma(nc.sync, g)
                nat_dma(nc.sync, t7, 0, t7, a_mid)
                nat_dma(nc.sync, t7, a_mid, t7, d_mid)
                nat_dma(nc.sync, t7, d_mid, t7, nat_cols)
            elif variant.startswith("hyb7"):
                # scalar ring kept short (3 issues) so ACT's queue frees
                # early; xt front-loaded so PE + diags finish mid-kernel.
                nc.scalar.dma_start(out=id_t, in_=ident[:, :])
                xt_dma(nc.scalar, 1)
                xt_dma(nc.scalar, 2)
                nat_dma(nc.sync, 0, 0, 0, nat_cols)
                nat_dma(nc.sync, 1, 0, 1, nat_cols)
                xt_dma(nc.sync, 0)
                nat_dma(nc.sync, 2, 0, 2, nat_cols)
                xt_dma(nc.sync, 3)
                nat_dma(nc.sync, 3, 0, 3, nat_cols)
                nat_dma(nc.sync, 4, 0, 4, nat_cols)
                nat_dma(nc.sync, 5, 0, 5, nat_cols)
                nat_dma(nc.sync, 6, 0, 6, nat_cols)
                nat_dma(nc.sync, t7, 0, t7, a_mid)
                nat_dma(nc.sync, t7, a_mid, t7, d_mid)
                nat_dma(nc.sync, t7, d_mid, t7, nat_cols)
            else:
                # two HWDGE rings in parallel: each ring FIFO, both
                # drive all 16 SDMA engines; alternate streams so
                # arrival order tracks consumption order.
                nc.scalar.dma_start(out=id_t, in_=ident[:, :])
                nat_dma(nc.sync, 0, 0, 0, nat_cols)
                nat_dma(nc.scalar, 1, 0, 1, nat_cols)
                xt_dma(nc.sync, 0)
                nat_dma(nc.scalar, 2, 0, 2, nat_cols)
                nat_dma(nc.sync, 3, 0, 3, nat_cols)
                xt_dma(nc.scalar, 1)
                nat_dma(nc.sync, 4, 0, 4, nat_cols)
                xt_dma(nc.scalar, 2)
                nat_dma(nc.sync, 5, 0, 5, nat_cols)
                xt_dma(nc.scalar, 3)
                nat_dma(nc.sync, 6, 0, 6, nat_cols)
                nat_dma(nc.sync, t7, 0, t7, a_mid)
                nat_dma(nc.sync, t7, a_mid, t7, d_mid)
                nat_dma(nc.sync, t7, d_mid, t7, nat_cols)

            # ── ScalarE ──
            warm = res.tile([_P, 1], f32, name="warm")
            nc.scalar.activation(warm, st[:, 0:1], AF.Square)
            for t in range(_TILES):
                scrA = sp.tile([_P, a_cols], f8, tag="scrA", name=f"sa_{t}")
                if t < _TILES - 1:
                    nc.scalar.activation(
                        scrA,
                        nat_ap(t, 0, a_cols),
                        AF.Square,
                        accum_out=st[:, t : t + 1],
                    )
                else:
                    nc.scalar.activation(
                        scrA[:, :a_mid],
                        nat_ap(t, 0, a_mid),
                        AF.Square,
                        accum_out=st[:, 7:8],
                    )
                    nc.scalar.activation(
                        scrA[:, a_mid:],
                        nat_ap(t, a_mid, a_cols),
                        AF.Square,
                        accum_out=st[:, 8:9],
                    )

            # ── TensorE ──
            for c in range(pe_chunks):
                g, l = c // GRP, c % GRP
                for rb in range(_TILES):
                    off = l * _RPC + rb * _P
                    sl = xtt[g][:, off : off + _P]
                    nc.tensor.matmul(
                        gram[rb][:, :_P],
                        sl,
                        sl,
                        start=(c == 0),
                        stop=(c == pe_chunks - 1),
                    )

            # ── VectorE ──
            def stt(t, lo, hi, sec):
                scrD = sp.tile([_P, hi - lo], f8, tag="scrD", name=f"sd_{sec}")
                nc.vector.scalar_tensor_tensor(
                    out=scrD,
                    in0=nat_ap(t, lo, hi),
                    scalar=1.0,
                    in1=nat_ap(t, lo, hi),
                    op0=ALU.mult,
                    op1=ALU.mult,
                    accum_out=st[:, sec : sec + 1],
                )

            for t in range(_TILES - 1):
                stt(t, a_cols, nat_cols, 9 + t)
            for rb in range(_TILES):
                dscr = sp.tile([_P, _P], bf16, tag="dscr", name=f"dg_{rb}")
                nc.vector.scalar_tensor_tensor(
                    out=dscr,
                    in0=gram[rb][:, :_P],
                    scalar=1.0,
                    in1=id_t,
                    op0=ALU.mult,
                    op1=ALU.mult,
                    accum_out=st[:, 18 + rb : 19 + rb],
                )
            nc.sync.dma_start(out=stats[:, 18:26], in_=st[:, 18:26])
            stt(_TILES - 1, a_cols, d_mid, 16)
            stt(_TILES - 1, d_mid, nat_cols, 17)
            nc.scalar.dma_start(out=stats[:, 0:9], in_=st[:, 0:9])
            nc.sync.dma_start(out=stats[:, 9:18], in_=st[:, 9:18])
    nc.finalize()
    return nc


def _build_hyb8(variant="hyb8"):
    """v8: two-ring issue (scalar-ring dma_starts interleaved between
    ACT's squares so ACT starts early), PE consumes xt groups in
    arrival order, balanced ring bytes, small final sub-chunks."""
    import concourse.bacc as bacc
    import concourse.mybir as mybir
    import concourse.tile as tile

    a_cols, d_cols, pe_chunks = _params(variant)
    nat_cols = a_cols + d_cols

    nc = bacc.Bacc(
        "TRN2", target_bir_lowering=False, debug=False, num_devices=_NCORES
    )
    f32 = mybir.dt.float32
    bf16 = mybir.dt.bfloat16
    f8 = mybir.dt.float8e4
    AF = mybir.ActivationFunctionType
    ALU = mybir.AluOpType

    xn = nc.dram_tensor(
        "xn2", [_P, _TILES * nat_cols], f8, kind="ExternalInput"
    )
    xt = nc.dram_tensor("xt", [_P, pe_chunks * _RPC], f8, kind="ExternalInput")
    ident = nc.dram_tensor("ident", [_P, _P], bf16, kind="ExternalInput")
    NSEC = 26
    stats = nc.dram_tensor("stats", [_P, NSEC], f32, kind="ExternalOutput")

    a_mid = (a_cols * 3 // 4 // _P) * _P
    d_mid = a_cols + (d_cols * 3 // 4 // _P) * _P

    GRP = 8
    n_grp = (pe_chunks + GRP - 1) // GRP

    with tile.TileContext(nc) as tc:
        with (
            tc.tile_pool(name="res", bufs=1) as res,
            tc.tile_pool(name="sp", bufs=2) as sp,
            tc.tile_pool(name="psum", bufs=1, space="PSUM") as psp,
        ):
            st = res.tile([_P, NSEC], f32, name="st")
            nc.vector.memset(st, 0.0)
            id_t = res.tile([_P, _P], bf16, name="id_t")
            gram = [
                psp.tile([_P, 512], f32, name=f"gram_{rb}")
                for rb in range(_TILES)
            ]
            xna = res.tile([_P, _TILES * nat_cols], f8, name="xna")
            xtt = [
                res.tile(
                    [_P, (min((g + 1) * GRP, pe_chunks) - g * GRP) * _RPC],
                    f8,
                    name=f"xt_{g}",
                )
                for g in range(n_grp)
            ]

            def nat_ap(t, lo, hi):
                return xna[:, t * nat_cols + lo : t * nat_cols + hi]

            def nat_dma(eng, lo_t, lo, hi_t, hi):
                a, b = lo_t * nat_cols + lo, hi_t * nat_cols + hi
                eng.dma_start(out=xna[:, a:b], in_=xn[:, a:b])

            def xt_dma(eng, g):
                lo = g * GRP * _RPC
                eng.dma_start(out=xtt[g], in_=xt[:, lo : lo + xtt[g].shape[1]])

            t7 = _TILES - 1
            warm = res.tile([_P, 1], f32, name="warm")

            def sq(t):
                scrA = sp.tile([_P, a_cols], f8, tag="scrA", name=f"sa_{t}")
                if t < t7:
                    nc.scalar.activation(
                        scrA,
                        nat_ap(t, 0, a_cols),
                        AF.Square,
                        accum_out=st[:, t : t + 1],
                    )
                else:
                    nc.scalar.activation(
                        scrA[:, :a_mid],
                        nat_ap(t, 0, a_mid),
                        AF.Square,
                        accum_out=st[:, 7:8],
                    )
                    nc.scalar.activation(
                        scrA[:, a_mid:],
                        nat_ap(t, a_mid, a_cols),
                        AF.Square,
                        accum_out=st[:, 8:9],
                    )

            # scalar ring: few issues, ALL before ACT's compute so the
            # queue frees early; sync ring carries the rest in
            # consumption order.
            nc.scalar.dma_start(out=id_t, in_=ident[:, :])
            nat_dma(nc.scalar, 1, 0, 1, nat_cols)
            xt_dma(nc.scalar, 1)
            if variant.endswith("b"):
                nat_dma(nc.scalar, 3, 0, 3, nat_cols)
                xt_dma(nc.scalar, 3)
            nat_dma(nc.sync, 0, 0, 0, nat_cols)
            xt_dma(nc.sync, 0)
            nat_dma(nc.sync, 2, 0, 2, nat_cols)
            xt_dma(nc.sync, 2)
            if not variant.endswith("b"):
                nat_dma(nc.sync, 3, 0, 3, nat_cols)
                xt_dma(nc.sync, 3)
            nat_dma(nc.sync, 4, 0, 4, nat_cols)
            nat_dma(nc.sync, 5, 0, 5, nat_cols)
            nat_dma(nc.sync, 6, 0, 6, nat_cols)
            nat_dma(nc.sync, t7, 0, t7, a_mid)
            nat_dma(nc.sync, t7, a_mid, t7, d_mid)
            nat_dma(nc.sync, t7, d_mid, t7, nat_cols)
            nc.scalar.activation(warm, st[:, 0:1], AF.Square)
            for t in range(_TILES):
                sq(t)

            # ── TensorE ──
            for c in range(pe_chunks):
                g, l = c // GRP, c % GRP
                for rb in range(_TILES):
                    off = l * _RPC + rb * _P
                    sl = xtt[g][:, off : off + _P]
                    nc.tensor.matmul(
                        gram[rb][:, :_P],
                        sl,
                        sl,
                        start=(c == 0),
                        stop=(c == pe_chunks - 1),
                    )

            # ── VectorE ──
            def stt(t, lo, hi, sec):
                scrD = sp.tile([_P, hi - lo], f8, tag="scrD", name=f"sd_{sec}")
                nc.vector.scalar_tensor_tensor(
                    out=scrD,
                    in0=nat_ap(t, lo, hi),
                    scalar=1.0,
                    in1=nat_ap(t, lo, hi),
                    op0=ALU.mult,
                    op1=ALU.mult,
                    accum_out=st[:, sec : sec + 1],
                )

            for t in range(_TILES - 1):
                stt(t, a_cols, nat_cols, 9 + t)
            for rb in range(_TILES):
                dscr = sp.tile([_P, _P], bf16, tag="dscr", name=f"dg_{rb}")
                nc.vector.scalar_tensor_tensor(
                    out=dscr,
                    in0=gram[rb][:, :_P],
                    scalar=1.0,
                    in1=id_t,
                    op0=ALU.mult,
                    op1=ALU.mult,
                    accum_out=st[:, 18 + rb : 19 + rb],
                )
            nc.sync.dma_start(out=stats[:, 18:26], in_=st[:, 18:26])
            stt(t7, a_cols, d_mid, 16)
            stt(t7, d_mid, nat_cols, 17)
            nc.scalar.dma_start(out=stats[:, 0:9], in_=st[:, 0:9])
            nc.sync.dma_start(out=stats[:, 9:18], in_=st[:, 9:18])
    nc.finalize()
    return nc


def _get_nc(variant=None):
    if variant is None:
        variant = VARIANT
    if variant not in _NC_CACHE:
        if variant.startswith("hyb8"):
            _NC_CACHE[variant] = _build_hyb8(variant)
        elif variant.startswith(("hyb5", "hyb6", "hyb7")):
            _NC_CACHE[variant] = _build_hyb5(variant)
        elif variant.startswith("hyb4"):
            _NC_CACHE[variant] = _build_hyb4(variant)
        elif variant.startswith("hyb3"):
            _NC_CACHE[variant] = _build_hyb3(variant)
        elif variant.startswith("hyb2"):
            _NC_CACHE[variant] = _build_hyb2(variant)
        else:
            _NC_CACHE[variant] = _build_bass(variant)
    return _NC_CACHE[variant]


def _make_in_maps(x, variant=None):
    """x: [BATCH, N] float32 -> per-core input dicts (fp8 cast here)."""
    import ml_dtypes

    if variant is None:
        variant = VARIANT
    a_cols, d_cols, pe_chunks = _params(variant)
    nat_cols = a_cols + d_cols
    x8 = x.astype(ml_dtypes.float8_e4m3)
    id128 = np.eye(_P, dtype=ml_dtypes.bfloat16)
    maps = []
    for i in range(_NCORES):
        xs = x8[i * _RPC : (i + 1) * _RPC]
        if variant.startswith(("hyb5", "hyb6", "hyb7", "hyb8")):
            # xn2[p, t*nat+c] = xs[t*128+p, c]
            nat = xs[:, :nat_cols].reshape(_TILES, _P, nat_cols)
            m = {
                "xn2": np.ascontiguousarray(
                    nat.transpose(1, 0, 2).reshape(_P, _TILES * nat_cols)
                )
            }
        else:
            m = {"xn": np.ascontiguousarray(xs[:, :nat_cols])}
        if pe_chunks:
            # packed[p, c*RPC + r] = xs[r, nat+c*128+p]
            pe = xs[:, nat_cols:].reshape(_RPC, pe_chunks, _P)
            m["xt"] = np.ascontiguousarray(
                pe.transpose(2, 1, 0).reshape(_P, pe_chunks * _RPC)
            )
            m["ident"] = id128
        maps.append(m)
    return maps


def _exact_p_y(xrows, yrows):
    """f64 exact solve of the knapsack dual for fallback rows."""
    xr = np.asarray(xrows, dtype=np.float64)
    if xr.ndim == 1:
        xr = xr[None, :]
    n = xr.shape[1]
    norm = np.maximum(np.sqrt((xr * xr).sum(1, keepdims=True)), 1e-12)
    e = xr / norm / _TAU
    lo = e.min(1) - _EPS
    hi = e.max(1) + _EPS * np.log(float(n))
    for _ in range(200):
        mid = 0.5 * (lo + hi)
        f = np.minimum(1.0, np.exp((e - mid[:, None]) / _EPS - 1.0)).sum(1)
        big = f > _K
        lo = np.where(big, mid, lo)
        hi = np.where(big, hi, mid)
    nu = 0.5 * (lo + hi)
    e_y = e[np.arange(e.shape[0]), yrows]
    return np.minimum(1.0, np.exp((e_y - nu) / _EPS - 1.0))


def kernel(x, y):
    from concourse.bass_utils import run_bass_kernel_spmd

    x = np.asarray(x, dtype=np.float32)
    y = np.asarray(y).astype(np.int64)
    assert x.shape == (_BATCH, _N)

    nc = _get_nc()
    in_maps = _make_in_maps(x)
    res = run_bass_kernel_spmd(nc, in_maps, core_ids=list(range(_NCORES)))

    parts = []
    for r in res.results:
        st = r["stats"].astype(np.float64)
        if VARIANT.startswith(("hyb2", "hyb3", "hyb4", "hyb5", "hyb6", "hyb7", "hyb8")):
            # cols 0-7 ACT (7=t7a), 8 ACT t7b, 9-16 DVE (16=t7a),
            # 17 DVE t7b, 18-25 PE blocks
            s2t = st[:, 0:8] + st[:, 9:17] + st[:, 18:26]  # [P, TILES]
            s2t[:, 7] += st[:, 8] + st[:, 17]
            s2 = s2t.T.reshape(-1)  # row t*128+p
        else:
            k = st.shape[1] // _TILES
            # S2 for row (t*128+p) = sum_sec st[p, sec*TILES + t]
            s2 = st.reshape(_P, k, _TILES).sum(axis=1).T.reshape(-1)
        parts.append(s2)
    S2 = np.concatenate(parts)  # [BATCH]

    rows = np.arange(_BATCH)
    x_y = x[rows, y].astype(np.float64)
    with np.errstate(all="ignore"):
        c = 1.0 / (np.sqrt(S2) * _TAU)
        e_y = x_y * c
        s = float(_N) + 0.5  # N + c*S1(dropped) + 0.5*c^2*S2 (== 0.5)
        p_y = np.minimum(1.0, _K * np.exp(e_y) / s)
        bad = ~(np.isfinite(p_y) & (S2 > 0))
    if bad.any():
        p_y[bad] = _exact_p_y(x[bad], y[bad])
    loss = np.mean(-np.log(p_y + 1e-8))
    return np.array(loss, dtype=np.float32)
